# revision 3
# baseline (speedup 1.0000x reference)
"""Trainium2 Bass kernel for nn_Decoder (scatter_memory).

Strategy: data-parallel over the batch dim (16 images / 8 cores = 2 per core).
Per core:
  - rotation matrices + SIREN hypernet computed on device (poly trig for accuracy)
  - per-point projections px/py and values via DVE pointwise ops over
    point-major [128, cols] planes
  - scatter-add via one-hot matmuls on the TensorEngine: per 128-point chunk,
    lhsT = onehot(hi=flat>>9) [128,128] fp16, rhs = onehot(lo=flat&511)*v
    [128,512] fp16, accumulated into a PSUM bank [128,512] = the 256x256 image
  - gaussian blur + rfft2 * ctf + irfft2 as matmuls against precomputed
    constant (blur-folded) DFT matrices
"""

import os
import sys

import numpy as np

_REPO = "/opt/trn_rl_repo"
if _REPO not in sys.path:
    sys.path.insert(0, _REPO)

B, LAT, N, XS = 16, 8, 500000, 256
W0_FIRST = 30.0
P = 128
N_CORES = 8
B_LOC = B // N_CORES           # images per core
COLS = -(-N // P)              # 3907 point columns per partition
NPAD = P * COLS                # 500096
ST_COLS = 512                  # supertile width (point columns)
MAGIC = 12582912.0             # 1.5 * 2**23 : float32 round-to-nearest-even

# ---------------------------------------------------------------------------
# host-side constants
# ---------------------------------------------------------------------------

def _gauss_kernel():
    x = np.arange(-3, 4, dtype=np.float64)
    k = np.exp(-0.5 * x * x)
    return k / k.sum()


def _post_matrices():
    """Blur-folded DFT matrices, arranged as matmul lhsT tiles.

    Returns [10, 128, 512] float32: A_r A_i C_r C_i C_mi V_r V_i V_mi Wy_r Wy_mi
    """
    k = _gauss_kernel()
    i = np.arange(XS)
    off = i[:, None] - i[None, :]
    Kb = np.where(np.abs(off) <= 3, k[np.clip(off + 3, 0, 6)], 0.0)
    ang = i[:, None] * i[None, :] * (-2j * np.pi / XS)
    W = np.exp(ang)            # [k, y]
    A = W @ Kb                 # fwd transform with blur folded [k, y]
    Winv = np.exp(-ang)        # [k, y] with e^{+2pi i k y}

    def arr_A(M):  # [p, j, kh, kl] = M[kh*128+kl, 2p+j]
        return np.ascontiguousarray(M.T.reshape(P, 2, 2, P).reshape(P, 512))

    def arr_C(M):  # [xp, xh, kh, kl] = M[kh*128+kl, xh*128+xp]
        return np.ascontiguousarray(
            M.T.reshape(2, P, 2, P).transpose(1, 0, 2, 3).reshape(P, 512))

    def arr_V(M):  # [kp, kh, xh, xl] = M[kh*128+kp, xh*128+xl]
        return np.ascontiguousarray(
            M.reshape(2, P, 2, P).transpose(1, 0, 2, 3).reshape(P, 512))

    mats = [
        arr_A(A.real), arr_A(A.imag),
        arr_C(A.real), arr_C(A.imag), arr_C(-A.imag),
        arr_V(Winv.real), arr_V(Winv.imag), arr_V(-Winv.imag),
        arr_V(Winv.real / (XS * XS)), arr_V(-Winv.imag / (XS * XS)),
    ]
    return np.stack(mats).astype(np.float32)


def _ctf_extend_T(ctf_b):
    """[256 ky, 129 kx] -> [128 kxp, 2 kxh, 256 ky] float32 (Hermitian mirror)."""
    ext = np.zeros((XS, XS), np.float32)       # [ky, kx]
    ext[:, :129] = ctf_b
    ky_idx = (-np.arange(XS)) % XS
    for kx in range(129, XS):
        ext[:, kx] = ctf_b[ky_idx, XS - kx]
    t = ext.T                                   # [kx, ky]
    return np.ascontiguousarray(t.reshape(2, P, XS).transpose(1, 0, 2))


# ---------------------------------------------------------------------------
# tile drain workaround: walrus here accepts only 1 sem wait per instruction
# ---------------------------------------------------------------------------

_PATCHED = False

def _patch_tile_drain():
    global _PATCHED
    if _PATCHED:
        return
    _PATCHED = True
    import concourse.tile as tile_mod
    from concourse.vector_clock import ScopedClock
    from concourse import mybir

    def _drain_and_barrier_split(self, tick_clock, wait_clock):
        nc = self.nc
        drain_inst = nc.sync.drain()
        wait_clock.add_sem_waits(
            drain_inst.ins, ScopedClock({None: tick_clock.global_clock}))
        si = drain_inst.ins.sync_info
        if si is not None and si.on_wait and len(si.on_wait) > 1:
            waits = list(si.on_wait)
            si.on_wait = waits[:1]
            for i in range(1, len(waits)):
                extra = nc.sync.drain()
                esi = extra.ins.sync_info
                if esi is None:
                    extra.ins.sync_info = mybir.SyncInfo(
                        on_wait=[waits[i]], on_update=[])
                else:
                    esi.on_wait = [waits[i]]
        nc.all_engine_barrier()
        assert self.sems is not None
        popped = nc._tile_sem_poison_stack.pop()
        assert popped is self._sem_poison
        nc.clear_and_free_semaphores(list(self.sems.allocated().values()))
        nc.all_engine_barrier()

    tile_mod.TileContext._drain_and_barrier = _drain_and_barrier_split


def _split_sync_waits(nc):
    """walrus here allows only one sem wait per instruction; hoist extras
    onto same-engine NOPs inserted immediately before."""
    from concourse import mybir
    for f in nc.m.functions:
        for bb in f.blocks:
            il = bb.instructions
            out_list = []
            changed = False
            for ins in il:
                si = getattr(ins, "sync_info", None)
                if si is not None and si.on_wait and len(si.on_wait) > 1:
                    waits = list(si.on_wait)
                    for w_ in waits[:-1]:
                        nop = mybir.InstNoOp(
                            name=f"wsplit-{nc.next_id()}", engine=ins.engine,
                            ins=[], outs=[],
                            sync_info=mybir.SyncInfo(on_wait=[w_], on_update=[]))
                        try:
                            nc.register_instruction(nop, overwrite=True)
                        except Exception:
                            pass
                        out_list.append(nop)
                    si.on_wait = waits[-1:]
                    changed = True
                out_list.append(ins)
            if changed:
                bb.instructions = out_list


# ---------------------------------------------------------------------------
# device program
# ---------------------------------------------------------------------------

def build_program(cols=COLS, st_cols=ST_COLS, debug=False):
    _patch_tile_drain()
    from concourse import bass, mybir
    from concourse.tile import TileContext
    from contextlib import ExitStack

    f32 = mybir.dt.float32
    f16 = mybir.dt.float16
    Alu = mybir.AluOpType
    Act = mybir.ActivationFunctionType

    nc = bass.Bass("TRN2", target_bir_lowering=False, debug=False,
                   num_devices=N_CORES)

    # ---- dram parameters -------------------------------------------------
    packed = nc.declare_dram_parameter("packed", [P, 13, cols], f32, isOutput=False)
    rows_flat = nc.declare_dram_parameter("rows_flat", [1, 6], f32, isOutput=False)
    shifts_flat = nc.declare_dram_parameter("shifts_flat", [1, 4], f32, isOutput=False)
    latentT = nc.declare_dram_parameter("latentT", [LAT, B_LOC], f32, isOutput=False)
    wmats = nc.declare_dram_parameter("wmats", [LAT, 4 * LAT], f32, isOutput=False)
    bvecs = nc.declare_dram_parameter("bvecs", [LAT, 4], f32, isOutput=False)
    ctf_arr = nc.declare_dram_parameter("ctf_arr", [P, B_LOC * 512], f32, isOutput=False)
    iota512 = nc.declare_dram_parameter("iota512", [P, 512], f16, isOutput=False)
    iota128 = nc.declare_dram_parameter("iota128", [P, P], f16, isOutput=False)
    ones_row = nc.declare_dram_parameter("ones_row", [1, P], f32, isOutput=False)
    ident = nc.declare_dram_parameter("ident", [P, P], f32, isOutput=False)
    postmats = nc.declare_dram_parameter("postmats", [P, 10, 512], f32, isOutput=False)
    out = nc.declare_dram_parameter("out", [B_LOC, XS, XS], f32, isOutput=True)
    if debug:
        dbg_bc = nc.declare_dram_parameter("dbg_bc", [P, 32], f32, isOutput=True)
        dbg_pl = nc.declare_dram_parameter("dbg_pl", [5, P, 64], f32, isOutput=True)
        dbg_img = nc.declare_dram_parameter("dbg_img", [B_LOC, P, 512], f32, isOutput=True)

    n_st = -(-cols // st_cols)

    with TileContext(nc, num_cores=N_CORES) as tc, ExitStack() as ctx:
        cpool = ctx.enter_context(tc.tile_pool(name="const", bufs=1))
        spool = ctx.enter_context(tc.tile_pool(name="scal", bufs=1))
        ppool = ctx.enter_context(tc.tile_pool(name="psum_s", bufs=2, space="PSUM"))
        sppool = ctx.enter_context(tc.tile_pool(name="psum_t", bufs=2, space="PSUM"))
        inpool = ctx.enter_context(tc.tile_pool(name="inp", bufs=2))
        plpool = ctx.enter_context(tc.tile_pool(name="plane", bufs=2))
        ohpool = ctx.enter_context(tc.tile_pool(name="oh", bufs=6))
        popool = ctx.enter_context(tc.tile_pool(name="post", bufs=1))
        pppool = ctx.enter_context(tc.tile_pool(name="psum_p", bufs=3, space="PSUM"))

        # ---- constants to SBUF ------------------------------------------
        io512 = cpool.tile([P, 512], f16)
        nc.sync.dma_start(out=io512[:], in_=iota512[:])
        io128 = cpool.tile([P, P], f16)
        nc.sync.dma_start(out=io128[:], in_=iota128[:])
        onesr = cpool.tile([1, P], f32)
        nc.sync.dma_start(out=onesr[:], in_=ones_row[:])
        idn = cpool.tile([P, P], f32)
        nc.sync.dma_start(out=idn[:], in_=ident[:])
        pm = cpool.tile([P, 10 * 512], f32)
        nc.sync.dma_start(out=pm[:], in_=postmats[:])
        ctf_sb = cpool.tile([P, B_LOC * 512], f32)
        nc.sync.dma_start(out=ctf_sb[:], in_=ctf_arr[:])
        rowsf = spool.tile([1, 6], f32)
        nc.sync.dma_start(out=rowsf[:], in_=rows_flat[:])
        shf = spool.tile([1, 4], f32)
        nc.sync.dma_start(out=shf[:], in_=shifts_flat[:])
        latT = spool.tile([LAT, B_LOC], f32)
        nc.sync.dma_start(out=latT[:], in_=latentT[:])
        wm = spool.tile([LAT, 4 * LAT], f32)
        nc.sync.dma_start(out=wm[:], in_=wmats[:])
        bv = spool.tile([LAT, 4], f32)
        nc.sync.dma_start(out=bv[:], in_=bvecs[:])

        def pmat(k, a, b):
            # lhsT slice of postmats matrix k, sub-block (a, b) [128, 128]
            return pm[:, k * 512 + a * 256 + b * 128: k * 512 + a * 256 + (b + 1) * 128]

        TT = nc.vector.tensor_tensor
        TS = nc.vector.tensor_scalar
        STT = nc.vector.scalar_tensor_tensor

        # ---- trig: sin/cos of the 6 euler angles (poly, ~1ulp) ----------
        def trig(x):  # x: [1, n] f32 tile -> (sin, cos) tiles
            n = x.shape[1]
            t = spool.tile([1, n], f32, tag="trig_t")
            q = spool.tile([1, n], f32, tag="trig_q")
            TS(out=t[:], in0=x[:], scalar1=float(2.0 / np.pi), scalar2=None, op0=Alu.mult)
            TS(out=q[:], in0=t[:], scalar1=MAGIC, scalar2=MAGIC, op0=Alu.add, op1=Alu.subtract)
            PIO2_HI = 1.57079601287841796875
            PIO2_LO = float(np.pi / 2 - PIO2_HI)
            r = spool.tile([1, n], f32, tag="trig_r")
            STT(out=r[:], in0=q[:], scalar=-PIO2_HI, in1=x[:], op0=Alu.mult, op1=Alu.add)
            STT(out=r[:], in0=q[:], scalar=-PIO2_LO, in1=r[:], op0=Alu.mult, op1=Alu.add)
            r2 = spool.tile([1, n], f32, tag="trig_r2")
            TT(out=r2[:], in0=r[:], in1=r[:], op=Alu.mult)
            # sin poly
            S = [-1.6666667163e-01, 8.3333337680e-03, -1.9841270114e-04,
                 2.7557314297e-06, -2.5050759689e-08]
            p = spool.tile([1, n], f32, tag="trig_p")
            TS(out=p[:], in0=r2[:], scalar1=S[4], scalar2=S[3], op0=Alu.mult, op1=Alu.add)
            for cf in (S[2], S[1], S[0]):
                TT(out=p[:], in0=p[:], in1=r2[:], op=Alu.mult)
                TS(out=p[:], in0=p[:], scalar1=cf, scalar2=None, op0=Alu.add)
            r3 = spool.tile([1, n], f32, tag="trig_r3")
            TT(out=r3[:], in0=r2[:], in1=r[:], op=Alu.mult)
            sp = spool.tile([1, n], f32, tag="trig_sp")
            TT(out=sp[:], in0=p[:], in1=r3[:], op=Alu.mult)
            TT(out=sp[:], in0=sp[:], in1=r[:], op=Alu.add)
            # cos poly
            C = [4.1666667908e-02, -1.3888889225e-03, 2.4801587642e-05,
                 -2.7557314297e-07]
            cpl = spool.tile([1, n], f32, tag="trig_cp")
            TS(out=cpl[:], in0=r2[:], scalar1=C[3], scalar2=C[2], op0=Alu.mult, op1=Alu.add)
            for cf in (C[1], C[0]):
                TT(out=cpl[:], in0=cpl[:], in1=r2[:], op=Alu.mult)
                TS(out=cpl[:], in0=cpl[:], scalar1=cf, scalar2=None, op0=Alu.add)
            TT(out=cpl[:], in0=cpl[:], in1=r2[:], op=Alu.mult)
            TS(out=cpl[:], in0=cpl[:], scalar1=-0.5, scalar2=None, op0=Alu.add)
            TT(out=cpl[:], in0=cpl[:], in1=r2[:], op=Alu.mult)
            cp = spool.tile([1, n], f32, tag="trig_cpf")
            TS(out=cp[:], in0=cpl[:], scalar1=1.0, scalar2=None, op0=Alu.add)
            # quadrant select: qm = q + 4*(q<0) ; masks
            neg = spool.tile([1, n], f32, tag="trig_neg")
            TS(out=neg[:], in0=q[:], scalar1=0.0, scalar2=None, op0=Alu.is_lt)
            qm = spool.tile([1, n], f32, tag="trig_qm")
            STT(out=qm[:], in0=neg[:], scalar=4.0, in1=q[:], op0=Alu.mult, op1=Alu.add)
            sres = spool.tile([1, n], f32, tag="trig_sres")
            cres = spool.tile([1, n], f32, tag="trig_cres")
            m = spool.tile([1, n], f32, tag="trig_m")
            tm = spool.tile([1, n], f32, tag="trig_tm")
            # sin = m0*sp + m1*cp - m2*sp - m3*cp ; cos = m0*cp - m1*sp - m2*cp + m3*sp
            TS(out=m[:], in0=qm[:], scalar1=0.0, scalar2=None, op0=Alu.is_equal)
            TT(out=sres[:], in0=m[:], in1=sp[:], op=Alu.mult)
            TT(out=cres[:], in0=m[:], in1=cp[:], op=Alu.mult)
            TS(out=m[:], in0=qm[:], scalar1=1.0, scalar2=None, op0=Alu.is_equal)
            TT(out=tm[:], in0=m[:], in1=cp[:], op=Alu.mult)
            TT(out=sres[:], in0=sres[:], in1=tm[:], op=Alu.add)
            TT(out=tm[:], in0=m[:], in1=sp[:], op=Alu.mult)
            TT(out=cres[:], in0=cres[:], in1=tm[:], op=Alu.subtract)
            TS(out=m[:], in0=qm[:], scalar1=2.0, scalar2=None, op0=Alu.is_equal)
            TT(out=tm[:], in0=m[:], in1=sp[:], op=Alu.mult)
            TT(out=sres[:], in0=sres[:], in1=tm[:], op=Alu.subtract)
            TT(out=tm[:], in0=m[:], in1=cp[:], op=Alu.mult)
            TT(out=cres[:], in0=cres[:], in1=tm[:], op=Alu.subtract)
            TS(out=m[:], in0=qm[:], scalar1=3.0, scalar2=None, op0=Alu.is_equal)
            TT(out=tm[:], in0=m[:], in1=cp[:], op=Alu.mult)
            TT(out=sres[:], in0=sres[:], in1=tm[:], op=Alu.subtract)
            TT(out=tm[:], in0=m[:], in1=sp[:], op=Alu.mult)
            TT(out=cres[:], in0=cres[:], in1=tm[:], op=Alu.add)
            return sres, cres

        sinv, cosv = trig(rowsf)   # [1, 6]: cols = (img, angle)

        # ---- SIREN in [LAT, B_LOC] layout -------------------------------
        def sin_reduced(dst, z_sb, bias_col, scale):
            """dst = sin(scale*z + bias) with range reduction; all [LAT, B_LOC]."""
            t = spool.tile([LAT, B_LOC], f32, tag="sir_t")
            if bias_col is not None:
                TS(out=t[:], in0=z_sb[:], scalar1=bias_col, scalar2=float(scale),
                   op0=Alu.add, op1=Alu.mult)
            else:
                TS(out=t[:], in0=z_sb[:], scalar1=float(scale), scalar2=None, op0=Alu.mult)
            u = spool.tile([LAT, B_LOC], f32, tag="sir_u")
            TS(out=u[:], in0=t[:], scalar1=float(1.0 / (2 * np.pi)), scalar2=None, op0=Alu.mult)
            k = spool.tile([LAT, B_LOC], f32, tag="sir_k")
            TS(out=k[:], in0=u[:], scalar1=MAGIC, scalar2=MAGIC, op0=Alu.add, op1=Alu.subtract)
            r = spool.tile([LAT, B_LOC], f32, tag="sir_r")
            STT(out=r[:], in0=k[:], scalar=float(-2 * np.pi), in1=t[:], op0=Alu.mult, op1=Alu.add)
            nc.scalar.activation(out=dst[:], in_=r[:], func=Act.Sin)

        h = spool.tile([LAT, B_LOC], f32, tag="h")
        zp = sppool.tile([LAT, B_LOC], f32, tag="sp")
        nc.tensor.matmul(out=zp[:], lhsT=wm[:, 0:LAT], rhs=latT[:], start=True, stop=True)
        z_sb = spool.tile([LAT, B_LOC], f32, tag="z_sb")
        nc.scalar.copy(out=z_sb[:], in_=zp[:])
        sin_reduced(h, z_sb, bv[:, 0:1], W0_FIRST)
        for li in range(1, 4):
            zp2 = sppool.tile([LAT, B_LOC], f32, tag="sp")
            nc.tensor.matmul(out=zp2[:], lhsT=wm[:, li * LAT:(li + 1) * LAT],
                             rhs=h[:], start=True, stop=True)
            nc.scalar.copy(out=z_sb[:], in_=zp2[:])
            sn = spool.tile([LAT, B_LOC], f32, tag="sir_sn")
            sin_reduced(sn, z_sb, bv[:, li:li + 1], 1.0)
            TT(out=h[:], in0=h[:], in1=sn[:], op=Alu.add)

        # ---- assemble scalar row s [1, 16*B_LOC] ------------------------
        NSC = 16
        s = spool.tile([1, NSC * B_LOC], f32, tag="s_row")

        def ang(b_, k_):  # AP helpers into sinv/cosv columns
            return (3 * b_ + k_, 3 * b_ + k_ + 1)

        t1 = spool.tile([1, 1], f32, tag="t1")
        t2 = spool.tile([1, 1], f32, tag="t2")
        for b_ in range(B_LOC):
            o = NSC * b_
            ca = cosv[:, 3 * b_:3 * b_ + 1]; sa = sinv[:, 3 * b_:3 * b_ + 1]
            cb = cosv[:, 3 * b_ + 1:3 * b_ + 2]; sb = sinv[:, 3 * b_ + 1:3 * b_ + 2]
            cg = cosv[:, 3 * b_ + 2:3 * b_ + 3]; sg = sinv[:, 3 * b_ + 2:3 * b_ + 3]
            # R00 = cg*cb*ca - sg*sa
            TT(out=t1[:], in0=cg, in1=cb, op=Alu.mult)
            TT(out=t1[:], in0=t1[:], in1=ca, op=Alu.mult)
            TT(out=t2[:], in0=sg, in1=sa, op=Alu.mult)
            TT(out=s[:, o + 0:o + 1], in0=t1[:], in1=t2[:], op=Alu.subtract)
            # R01 = cg*cb*sa + sg*ca
            TT(out=t1[:], in0=cg, in1=cb, op=Alu.mult)
            TT(out=t1[:], in0=t1[:], in1=sa, op=Alu.mult)
            TT(out=t2[:], in0=sg, in1=ca, op=Alu.mult)
            TT(out=s[:, o + 1:o + 2], in0=t1[:], in1=t2[:], op=Alu.add)
            # R02 = -cg*sb
            TT(out=t1[:], in0=cg, in1=sb, op=Alu.mult)
            TS(out=s[:, o + 2:o + 3], in0=t1[:], scalar1=-1.0, scalar2=None, op0=Alu.mult)
            # sx + 128
            TS(out=s[:, o + 3:o + 4], in0=shf[:, 2 * b_:2 * b_ + 1],
               scalar1=float(XS // 2), scalar2=None, op0=Alu.add)
            # R10 = -(sg*cb*ca + cg*sa)
            TT(out=t1[:], in0=sg, in1=cb, op=Alu.mult)
            TT(out=t1[:], in0=t1[:], in1=ca, op=Alu.mult)
            TT(out=t2[:], in0=cg, in1=sa, op=Alu.mult)
            TT(out=t1[:], in0=t1[:], in1=t2[:], op=Alu.add)
            TS(out=s[:, o + 4:o + 5], in0=t1[:], scalar1=-1.0, scalar2=None, op0=Alu.mult)
            # R11 = cg*ca - sg*cb*sa
            TT(out=t1[:], in0=sg, in1=cb, op=Alu.mult)
            TT(out=t1[:], in0=t1[:], in1=sa, op=Alu.mult)
            TT(out=t2[:], in0=cg, in1=ca, op=Alu.mult)
            TT(out=s[:, o + 5:o + 6], in0=t2[:], in1=t1[:], op=Alu.subtract)
            # R12 = sg*sb
            TT(out=s[:, o + 6:o + 7], in0=sg, in1=sb, op=Alu.mult)
            # sy + 128
            TS(out=s[:, o + 7:o + 8], in0=shf[:, 2 * b_ + 1:2 * b_ + 2],
               scalar1=float(XS // 2), scalar2=None, op0=Alu.add)
            # h columns via transpose of h[:, b] -> [1, LAT]
            hp = sppool.tile([1, LAT], f32, tag="sp")
            nc.tensor.transpose(out=hp[:], in_=h[:, b_:b_ + 1], identity=idn[:LAT, :LAT])
            nc.vector.tensor_copy(out=s[:, o + 8:o + 16], in_=hp[:])

        # broadcast s across partitions
        bps = sppool.tile([P, NSC * B_LOC], f32, tag="sp")
        nc.tensor.matmul(out=bps[:], lhsT=onesr[:], rhs=s[:], start=True, stop=True)
        bc = spool.tile([P, NSC * B_LOC], f32, tag="bcast")
        nc.vector.tensor_copy(out=bc[:], in_=bps[:])
        if debug:
            nc.sync.dma_start(out=dbg_bc[:], in_=bc[:])

        def bcol(b_, k_):
            return bc[:, NSC * b_ + k_: NSC * b_ + k_ + 1]

        # ---- scatter accumulators ---------------------------------------
        img_ps = [ppool.tile([P, 512], f32, name=f"img_ps{_b}", tag="img_ps") for _b in range(B_LOC)]
        n_chunks_total = cols

        # ---- main loop ---------------------------------------------------
        done_chunks = 0
        for st in range(n_st):
            w = min(st_cols, cols - st * st_cols)
            inp = inpool.tile([P, 13, st_cols], f32, tag="inp")
            nc.sync.dma_start(out=inp[:, :, :w], in_=packed[:, :, st * st_cols: st * st_cols + w])
            cx = inp[:, 0, :w]; cy = inp[:, 1, :w]; cz = inp[:, 2, :w]
            vals_p = inp[:, 3, :w]; bd_p = inp[:, 4, :w]

            vb = plpool.tile([P, st_cols], f32, tag="vb")
            TT(out=vb[:, :w], in0=vals_p, in1=bd_p, op=Alu.add)

            lo16 = []; hi16 = []; v16 = []
            for b_ in range(B_LOC):
                # px = cx*R00 + cy*R01 + cz*R02 + (sx+128)
                px = plpool.tile([P, st_cols], f32, tag="px")
                TS(out=px[:, :w], in0=cx, scalar1=bcol(b_, 0), scalar2=None, op0=Alu.mult)
                STT(out=px[:, :w], in0=cy, scalar=bcol(b_, 1), in1=px[:, :w], op0=Alu.mult, op1=Alu.add)
                STT(out=px[:, :w], in0=cz, scalar=bcol(b_, 2), in1=px[:, :w], op0=Alu.mult, op1=Alu.add)
                TS(out=px[:, :w], in0=px[:, :w], scalar1=bcol(b_, 3), scalar2=None, op0=Alu.add)
                py = plpool.tile([P, st_cols], f32, tag="py")
                TS(out=py[:, :w], in0=cx, scalar1=bcol(b_, 4), scalar2=None, op0=Alu.mult)
                STT(out=py[:, :w], in0=cy, scalar=bcol(b_, 5), in1=py[:, :w], op0=Alu.mult, op1=Alu.add)
                STT(out=py[:, :w], in0=cz, scalar=bcol(b_, 6), in1=py[:, :w], op0=Alu.mult, op1=Alu.add)
                TS(out=py[:, :w], in0=py[:, :w], scalar1=bcol(b_, 7), scalar2=None, op0=Alu.add)
                # round + clip
                TS(out=px[:, :w], in0=px[:, :w], scalar1=MAGIC, scalar2=MAGIC, op0=Alu.add, op1=Alu.subtract)
                TS(out=px[:, :w], in0=px[:, :w], scalar1=0.0, scalar2=255.0, op0=Alu.max, op1=Alu.min)
                TS(out=py[:, :w], in0=py[:, :w], scalar1=MAGIC, scalar2=MAGIC, op0=Alu.add, op1=Alu.subtract)
                TS(out=py[:, :w], in0=py[:, :w], scalar1=0.0, scalar2=255.0, op0=Alu.max, op1=Alu.min)
                # hi = floor(py/2) = round(py*0.5 - 0.25) ; m = py - 2*hi ; lo = m*256 + px
                hi = plpool.tile([P, st_cols], f32, tag="hi")
                TS(out=hi[:, :w], in0=py[:, :w], scalar1=0.5, scalar2=-0.25, op0=Alu.mult, op1=Alu.add)
                TS(out=hi[:, :w], in0=hi[:, :w], scalar1=MAGIC, scalar2=MAGIC, op0=Alu.add, op1=Alu.subtract)
                m = plpool.tile([P, st_cols], f32, tag="m")
                STT(out=m[:, :w], in0=hi[:, :w], scalar=-2.0, in1=py[:, :w], op0=Alu.mult, op1=Alu.add)
                lo_t = ohpool.tile([P, st_cols], f32, tag="lo16")
                STT(out=lo_t[:, :w], in0=m[:, :w], scalar=256.0, in1=px[:, :w], op0=Alu.mult, op1=Alu.add)
                hi_t = ohpool.tile([P, st_cols], f32, tag="hi16")
                nc.vector.tensor_copy(out=hi_t[:, :w], in_=hi[:, :w])
                # v = vb + sum_l h[l]*Wd_l
                acc = plpool.tile([P, st_cols], f32, tag="acc")
                STT(out=acc[:, :w], in0=inp[:, 5, :w], scalar=bcol(b_, 8), in1=vb[:, :w],
                    op0=Alu.mult, op1=Alu.add)
                for l_ in range(1, LAT):
                    STT(out=acc[:, :w], in0=inp[:, 5 + l_, :w], scalar=bcol(b_, 8 + l_),
                        in1=acc[:, :w], op0=Alu.mult, op1=Alu.add)
                v_t = ohpool.tile([P, st_cols], f32, tag="v16")
                nc.vector.tensor_copy(out=v_t[:, :w], in_=acc[:, :w])
                lo16.append(lo_t); hi16.append(hi_t); v16.append(v_t)
                if debug and st == 0 and b_ == 0:
                    dw = min(64, w)
                    nc.sync.dma_start(out=dbg_pl[0, :, :dw], in_=px[:, :dw])
                    nc.sync.dma_start(out=dbg_pl[1, :, :dw], in_=py[:, :dw])
                    nc.sync.dma_start(out=dbg_pl[2, :, :dw], in_=lo_t[:, :dw])
                    nc.sync.dma_start(out=dbg_pl[3, :, :dw], in_=hi_t[:, :dw])
                    nc.sync.dma_start(out=dbg_pl[4, :, :dw], in_=v_t[:, :dw])

            for c in range(w):
                first = (done_chunks == 0)
                last = (done_chunks == n_chunks_total - 1)
                for b_ in range(B_LOC):
                    oh5 = ohpool.tile([P, 512], f16, tag="oh5", bufs=8)
                    TS(out=oh5[:], in0=io512[:], scalar1=lo16[b_][:, c:c + 1],
                       scalar2=v16[b_][:, c:c + 1], op0=Alu.is_equal, op1=Alu.mult)
                    oh1 = ohpool.tile([P, P], f16, tag="oh1", bufs=8)
                    # balance one-hot generation: Pool (gpsimd) carries most
                    # oh1s; DVE keeps 1 in 7 so DVE and Pool finish together
                    oh1_eng = nc.vector if (done_chunks % 7 == 3) else nc.gpsimd
                    oh1_eng.tensor_scalar(
                        out=oh1[:], in0=io128[:], scalar1=hi16[b_][:, c:c + 1],
                        scalar2=None, op0=Alu.is_equal)
                    nc.tensor.matmul(out=img_ps[b_][:], lhsT=oh1[:], rhs=oh5[:],
                                     start=first, stop=last, skip_group_check=True)
                done_chunks += 1

        # ---- post-processing per image ----------------------------------
        for b_ in range(B_LOC):
            img_sb = popool.tile([P, 512], f32, tag="img_sb")
            nc.scalar.copy(out=img_sb[:], in_=img_ps[b_][:])
            if debug:
                nc.sync.dma_start(out=dbg_img[b_], in_=img_sb[:])
            # Y-pass: Ty[c][kh] = sum_j A_c(j, kh)^T @ img[:, j*256:...]
            Ty = []
            for ci in range(2):           # 0: real, 1: imag
                tysb = popool.tile([P, 512], f32, tag=f"ty{ci}")
                for kh in range(2):
                    tp = pppool.tile([P, 256], f32, tag="pp")
                    for j in range(2):
                        nc.tensor.matmul(out=tp[:], lhsT=pmat(ci, j, kh),
                                         rhs=img_sb[:, j * 256:(j + 1) * 256],
                                         start=(j == 0), stop=(j == 1))
                    nc.scalar.copy(out=tysb[:, kh * 256:(kh + 1) * 256], in_=tp[:])
                Ty.append(tysb)
            # transpose Ty -> TyT [x-part, ky-free]
            TyT = []
            for ci in range(2):
                ttsb = popool.tile([P, 512], f32, tag=f"tyt{ci}")
                for kh in range(2):
                    for xh in range(2):
                        tp = pppool.tile([P, P], f32, tag="pp")
                        nc.tensor.transpose(
                            out=tp[:], in_=Ty[ci][:, kh * 256 + xh * 128: kh * 256 + (xh + 1) * 128],
                            identity=idn[:])
                        nc.scalar.copy(
                            out=ttsb[:, xh * 256 + kh * 128: xh * 256 + (kh + 1) * 128], in_=tp[:])
                TyT.append(ttsb)
            # X-pass: F[oc][kxh] ; Fr = Cr@Tr - Ci@Ti ; Fi = Cr@Ti + Ci@Tr
            # then G = F * ctf
            G = []
            for oc in range(2):
                gsb = popool.tile([P, 512], f32, tag=f"g{oc}")
                terms = ([(2, 0), (4, 1)] if oc == 0 else [(2, 1), (3, 0)])
                for kxh in range(2):
                    fp = pppool.tile([P, 256], f32, tag="pp")
                    mm = 0
                    for (mk, src) in terms:
                        for xh in range(2):
                            nc.tensor.matmul(out=fp[:], lhsT=pmat(mk, xh, kxh),
                                             rhs=TyT[src][:, xh * 256:(xh + 1) * 256],
                                             start=(mm == 0), stop=(mm == 3))
                            mm += 1
                    TT(out=gsb[:, kxh * 256:(kxh + 1) * 256], in0=fp[:],
                       in1=ctf_sb[:, b_ * 512 + kxh * 256: b_ * 512 + (kxh + 1) * 256],
                       op=Alu.mult)
                G.append(gsb)
            # iX-pass: Z[oc][xh] ; Zr = Vr@Gr - Vi@Gi ; Zi = Vr@Gi + Vi@Gr
            Z = []
            for oc in range(2):
                zsb = popool.tile([P, 512], f32, tag=f"z{oc}")
                terms = ([(5, 0), (7, 1)] if oc == 0 else [(5, 1), (6, 0)])
                for xh in range(2):
                    zp_ = pppool.tile([P, 256], f32, tag="pp")
                    mm = 0
                    for (mk, src) in terms:
                        for kxh in range(2):
                            nc.tensor.matmul(out=zp_[:], lhsT=pmat(mk, kxh, xh),
                                             rhs=G[src][:, kxh * 256:(kxh + 1) * 256],
                                             start=(mm == 0), stop=(mm == 3))
                            mm += 1
                    nc.scalar.copy(out=zsb[:, xh * 256:(xh + 1) * 256], in_=zp_[:])
                Z.append(zsb)
            # transpose Z -> Zt[oc][kyh] [ky-part, x-free]
            Zt = [[], []]
            for oc in range(2):
                for kyh in range(2):
                    ztsb = popool.tile([P, 256], f32, tag=f"zt{oc}{kyh}")
                    for xh in range(2):
                        tp = pppool.tile([P, P], f32, tag="pp")
                        nc.tensor.transpose(
                            out=tp[:], in_=Z[oc][:, xh * 256 + kyh * 128: xh * 256 + (kyh + 1) * 128],
                            identity=idn[:])
                        nc.scalar.copy(out=ztsb[:, xh * 128:(xh + 1) * 128], in_=tp[:])
                    Zt[oc].append(ztsb)
            # iY-pass: out[yh] = sum_kyh (Wy_r@Ztr - Wy_i@Zti)
            for yh in range(2):
                op_ = pppool.tile([P, 256], f32, tag="pp")
                mm = 0
                for (mk, oc) in ((8, 0), (9, 1)):
                    for kyh in range(2):
                        nc.tensor.matmul(out=op_[:], lhsT=pmat(mk, kyh, yh),
                                         rhs=Zt[oc][kyh][:], start=(mm == 0), stop=(mm == 3))
                        mm += 1
                osb = popool.tile([P, 256], f32, tag="osb")
                nc.scalar.copy(out=osb[:], in_=op_[:])
                nc.sync.dma_start(out=out[b_, yh * P:(yh + 1) * P, :], in_=osb[:])

    _split_sync_waits(nc)
    return nc


# ---------------------------------------------------------------------------
# host wrapper
# ---------------------------------------------------------------------------

_PROG_CACHE = {}


def _get_program(cols, debug=False):
    key = (cols, debug)
    if key not in _PROG_CACHE:
        _PROG_CACHE[key] = build_program(cols=cols, debug=debug)
    return _PROG_CACHE[key]


def _marshal(inputs, cols=COLS):
    npad = P * cols
    n_use = min(N, npad)

    def plane(a):
        f = np.zeros(npad, np.float32)
        f[:n_use] = np.asarray(a, np.float32).ravel()[:n_use]
        return f.reshape(P, cols)

    coords = np.asarray(inputs["coords"], np.float32)
    packed = np.zeros((P, 13, cols), np.float32)
    packed[:, 0] = plane(coords[:n_use, 0])
    packed[:, 1] = plane(coords[:n_use, 1])
    packed[:, 2] = plane(coords[:n_use, 2])
    packed[:, 3] = plane(inputs["values"][:n_use])
    packed[:, 4] = plane(inputs["bd"][:n_use])
    Wd = np.asarray(inputs["Wd"], np.float32)
    for l_ in range(LAT):
        packed[:, 5 + l_] = plane(Wd[l_, :n_use])

    iota512 = np.tile(np.arange(512, dtype=np.float16), (P, 1))
    iota128 = np.tile(np.arange(P, dtype=np.float16), (P, 1))
    ones_row = np.ones((1, P), np.float32)
    ident = np.eye(P, dtype=np.float32)
    postmats = np.ascontiguousarray(_post_matrices().transpose(1, 0, 2))  # [128, 10, 512]
    wmats = np.concatenate([np.asarray(inputs[k], np.float32) for k in ("W0", "W1", "W2", "W3")], axis=1)
    bvecs = np.stack([np.asarray(inputs[k], np.float32) for k in ("b0", "b1", "b2", "b3")], axis=1)

    rows = np.asarray(inputs["rows"], np.float32)
    shifts = np.asarray(inputs["shifts"], np.float32)
    latent = np.asarray(inputs["latent"], np.float32)
    ctf = np.asarray(inputs["ctf"], np.float32)

    in_maps = []
    for core in range(N_CORES):
        bs = slice(core * B_LOC, (core + 1) * B_LOC)
        ctf_core = np.concatenate(
            [_ctf_extend_T(c).reshape(P, 512) for c in ctf[bs]], axis=1)
        in_maps.append({
            "packed": packed,
            "rows_flat": rows[bs].reshape(1, 6),
            "shifts_flat": shifts[bs].reshape(1, 4),
            "latentT": np.ascontiguousarray(latent[bs].T),
            "wmats": wmats,
            "bvecs": np.ascontiguousarray(bvecs),
            "ctf_arr": ctf_core,
            "iota512": iota512,
            "iota128": iota128,
            "ones_row": ones_row,
            "ident": ident,
            "postmats": postmats,
        })
    return in_maps


def run(inputs, cols=COLS, trace=False, debug=False):
    from concourse.bass_utils import run_bass_kernel_spmd
    nc = _get_program(cols, debug)
    in_maps = _marshal(inputs, cols)
    res = run_bass_kernel_spmd(nc, in_maps, list(range(N_CORES)), trace=trace)
    outs = [res.results[i]["out"] for i in range(N_CORES)]
    full = np.concatenate(outs, axis=0).astype(np.float32)
    return full, res


def kernel(**inputs):
    out, _ = run(inputs)
    return out



# revision 8
# speedup vs baseline: 3.2983x; 3.2983x over previous
"""Trainium2 Bass kernel for nn_Decoder (scatter_memory).

Strategy: data-parallel over the batch dim (16 images / 8 cores = 2 per core).
Per core:
  - rotation matrices + SIREN hypernet computed on device (poly trig for accuracy)
  - per-point projections px/py and values via DVE pointwise ops over
    point-major [128, cols] planes
  - scatter-add via one-hot matmuls on the TensorEngine: per 128-point chunk,
    lhsT = onehot(hi=flat>>9) [128,128] fp16, rhs = onehot(lo=flat&511)*v
    [128,512] fp16, accumulated into a PSUM bank [128,512] = the 256x256 image
  - gaussian blur + rfft2 * ctf + irfft2 as matmuls against precomputed
    constant (blur-folded) DFT matrices
"""

import os
import sys

import numpy as np

_REPO = "/opt/trn_rl_repo"
if _REPO not in sys.path:
    sys.path.insert(0, _REPO)

B, LAT, N, XS = 16, 8, 500000, 256
W0_FIRST = 30.0
P = 128
N_CORES = 8
B_LOC = B // N_CORES           # images per core
COLS = -(-N // P)              # 3907 point columns per partition
NPAD = P * COLS                # 500096
ST_COLS = 512                  # supertile width (point columns)
MAGIC = 12582912.0             # 1.5 * 2**23 : float32 round-to-nearest-even
ACT_MOD = 9                    # ACT engine takes len(ACT_RES)/ACT_MOD of the
ACT_RES = (1, 5)               # wide one-hots (offloads the DVE bottleneck)

# ---------------------------------------------------------------------------
# host-side constants
# ---------------------------------------------------------------------------

def _gauss_kernel():
    x = np.arange(-3, 4, dtype=np.float64)
    k = np.exp(-0.5 * x * x)
    return k / k.sum()


def _post_matrices():
    """Blur-folded DFT matrices, arranged as matmul lhsT tiles.

    Returns [10, 128, 512] float32: A_r A_i C_r C_i C_mi V_r V_i V_mi Wy_r Wy_mi
    """
    k = _gauss_kernel()
    i = np.arange(XS)
    off = i[:, None] - i[None, :]
    Kb = np.where(np.abs(off) <= 3, k[np.clip(off + 3, 0, 6)], 0.0)
    ang = i[:, None] * i[None, :] * (-2j * np.pi / XS)
    W = np.exp(ang)            # [k, y]
    A = W @ Kb                 # fwd transform with blur folded [k, y]
    Winv = np.exp(-ang)        # [k, y] with e^{+2pi i k y}

    def arr_A(M):  # [p, j, kh, kl] = M[kh*128+kl, 2p+j]
        return np.ascontiguousarray(M.T.reshape(P, 2, 2, P).reshape(P, 512))

    def arr_C(M):  # [xp, xh, kh, kl] = M[kh*128+kl, xh*128+xp]
        return np.ascontiguousarray(
            M.T.reshape(2, P, 2, P).transpose(1, 0, 2, 3).reshape(P, 512))

    def arr_V(M):  # [kp, kh, xh, xl] = M[kh*128+kp, xh*128+xl]
        return np.ascontiguousarray(
            M.reshape(2, P, 2, P).transpose(1, 0, 2, 3).reshape(P, 512))

    mats = [
        arr_A(A.real), arr_A(A.imag),
        arr_C(A.real), arr_C(A.imag), arr_C(-A.imag),
        arr_V(Winv.real), arr_V(Winv.imag), arr_V(-Winv.imag),
        arr_V(Winv.real / (XS * XS)), arr_V(-Winv.imag / (XS * XS)),
    ]
    return np.stack(mats).astype(np.float32)


def _ctf_extend_T(ctf_b):
    """[256 ky, 129 kx] -> [128 kxp, 2 kxh, 256 ky] float32 (Hermitian mirror)."""
    ext = np.zeros((XS, XS), np.float32)       # [ky, kx]
    ext[:, :129] = ctf_b
    ky_idx = (-np.arange(XS)) % XS
    for kx in range(129, XS):
        ext[:, kx] = ctf_b[ky_idx, XS - kx]
    t = ext.T                                   # [kx, ky]
    return np.ascontiguousarray(t.reshape(2, P, XS).transpose(1, 0, 2))


# ---------------------------------------------------------------------------
# tile drain workaround: walrus here accepts only 1 sem wait per instruction
# ---------------------------------------------------------------------------

_PATCHED = False

def _patch_tile_drain():
    global _PATCHED
    if _PATCHED:
        return
    _PATCHED = True
    import concourse.tile as tile_mod
    from concourse.vector_clock import ScopedClock
    from concourse import mybir

    def _drain_and_barrier_split(self, tick_clock, wait_clock):
        nc = self.nc
        drain_inst = nc.sync.drain()
        wait_clock.add_sem_waits(
            drain_inst.ins, ScopedClock({None: tick_clock.global_clock}))
        si = drain_inst.ins.sync_info
        if si is not None and si.on_wait and len(si.on_wait) > 1:
            waits = list(si.on_wait)
            si.on_wait = waits[:1]
            for i in range(1, len(waits)):
                extra = nc.sync.drain()
                esi = extra.ins.sync_info
                if esi is None:
                    extra.ins.sync_info = mybir.SyncInfo(
                        on_wait=[waits[i]], on_update=[])
                else:
                    esi.on_wait = [waits[i]]
        nc.all_engine_barrier()
        assert self.sems is not None
        popped = nc._tile_sem_poison_stack.pop()
        assert popped is self._sem_poison
        nc.clear_and_free_semaphores(list(self.sems.allocated().values()))
        nc.all_engine_barrier()

    tile_mod.TileContext._drain_and_barrier = _drain_and_barrier_split


def _split_sync_waits(nc):
    """walrus here allows only one sem wait per instruction; hoist extras
    onto same-engine NOPs inserted immediately before."""
    from concourse import mybir
    for f in nc.m.functions:
        for bb in f.blocks:
            il = bb.instructions
            out_list = []
            changed = False
            for ins in il:
                si = getattr(ins, "sync_info", None)
                if si is not None and si.on_wait and len(si.on_wait) > 1:
                    waits = list(si.on_wait)
                    for w_ in waits[:-1]:
                        nop = mybir.InstNoOp(
                            name=f"wsplit-{nc.next_id()}", engine=ins.engine,
                            ins=[], outs=[],
                            sync_info=mybir.SyncInfo(on_wait=[w_], on_update=[]))
                        try:
                            nc.register_instruction(nop, overwrite=True)
                        except Exception:
                            pass
                        out_list.append(nop)
                    si.on_wait = waits[-1:]
                    changed = True
                out_list.append(ins)
            if changed:
                bb.instructions = out_list


# ---------------------------------------------------------------------------
# device program
# ---------------------------------------------------------------------------

def build_program(cols=COLS, st_cols=ST_COLS, debug=False):
    _patch_tile_drain()
    from concourse import bass, mybir
    from concourse.tile import TileContext
    from contextlib import ExitStack

    f32 = mybir.dt.float32
    f16 = mybir.dt.float16
    Alu = mybir.AluOpType
    Act = mybir.ActivationFunctionType

    nc = bass.Bass("TRN2", target_bir_lowering=False, debug=False,
                   num_devices=N_CORES)

    # ---- dram parameters -------------------------------------------------
    packed = nc.declare_dram_parameter("packed", [P, 13, cols], f32, isOutput=False)
    rows_flat = nc.declare_dram_parameter("rows_flat", [1, 6], f32, isOutput=False)
    shifts_flat = nc.declare_dram_parameter("shifts_flat", [1, 4], f32, isOutput=False)
    latentT = nc.declare_dram_parameter("latentT", [LAT, B_LOC], f32, isOutput=False)
    wmats = nc.declare_dram_parameter("wmats", [LAT, 4 * LAT], f32, isOutput=False)
    bvecs = nc.declare_dram_parameter("bvecs", [LAT, 4], f32, isOutput=False)
    ctf_arr = nc.declare_dram_parameter("ctf_arr", [P, B_LOC * 512], f32, isOutput=False)
    iota512 = nc.declare_dram_parameter("iota512", [P, 512], f16, isOutput=False)
    iota128 = nc.declare_dram_parameter("iota128", [P, P], f16, isOutput=False)
    ones_row = nc.declare_dram_parameter("ones_row", [1, P], f32, isOutput=False)
    ident = nc.declare_dram_parameter("ident", [P, P], f32, isOutput=False)
    postmats = nc.declare_dram_parameter("postmats", [P, 10, 512], f32, isOutput=False)
    out = nc.declare_dram_parameter("out", [B_LOC, XS, XS], f32, isOutput=True)
    if debug:
        dbg_bc = nc.declare_dram_parameter("dbg_bc", [P, 32], f32, isOutput=True)
        dbg_pl = nc.declare_dram_parameter("dbg_pl", [5, P, 64], f32, isOutput=True)
        dbg_img = nc.declare_dram_parameter("dbg_img", [B_LOC, P, 512], f32, isOutput=True)

    n_st = -(-cols // st_cols)

    with TileContext(nc, num_cores=N_CORES) as tc, ExitStack() as ctx:
        cpool = ctx.enter_context(tc.tile_pool(name="const", bufs=1))
        spool = ctx.enter_context(tc.tile_pool(name="scal", bufs=1))
        ppool = ctx.enter_context(tc.tile_pool(name="psum_s", bufs=2, space="PSUM"))
        sppool = ctx.enter_context(tc.tile_pool(name="psum_t", bufs=2, space="PSUM"))
        inpool = ctx.enter_context(tc.tile_pool(name="inp", bufs=2))
        plpool = ctx.enter_context(tc.tile_pool(name="plane", bufs=2))
        ohpool = ctx.enter_context(tc.tile_pool(name="oh", bufs=6))
        popool = ctx.enter_context(tc.tile_pool(name="post", bufs=1))
        pppool = ctx.enter_context(tc.tile_pool(name="psum_p", bufs=3, space="PSUM"))

        # ---- constants to SBUF ------------------------------------------
        io512 = cpool.tile([P, 512], f16)
        nc.sync.dma_start(out=io512[:], in_=iota512[:])
        io128 = cpool.tile([P, P], f16)
        nc.sync.dma_start(out=io128[:], in_=iota128[:])
        onesr = cpool.tile([1, P], f32)
        nc.sync.dma_start(out=onesr[:], in_=ones_row[:])
        idn = cpool.tile([P, P], f32)
        nc.sync.dma_start(out=idn[:], in_=ident[:])
        pm = cpool.tile([P, 10 * 512], f32)
        nc.sync.dma_start(out=pm[:], in_=postmats[:])
        ctf_sb = cpool.tile([P, B_LOC * 512], f32)
        nc.sync.dma_start(out=ctf_sb[:], in_=ctf_arr[:])
        rowsf = spool.tile([1, 6], f32)
        nc.sync.dma_start(out=rowsf[:], in_=rows_flat[:])
        shf = spool.tile([1, 4], f32)
        nc.sync.dma_start(out=shf[:], in_=shifts_flat[:])
        latT = spool.tile([LAT, B_LOC], f32)
        nc.sync.dma_start(out=latT[:], in_=latentT[:])
        wm = spool.tile([LAT, 4 * LAT], f32)
        nc.sync.dma_start(out=wm[:], in_=wmats[:])
        bv = spool.tile([LAT, 4], f32)
        nc.sync.dma_start(out=bv[:], in_=bvecs[:])

        def pmat(k, a, b):
            # lhsT slice of postmats matrix k, sub-block (a, b) [128, 128]
            return pm[:, k * 512 + a * 256 + b * 128: k * 512 + a * 256 + (b + 1) * 128]

        TT = nc.vector.tensor_tensor
        TS = nc.vector.tensor_scalar
        STT = nc.vector.scalar_tensor_tensor

        # ---- trig: sin/cos of the 6 euler angles (poly, ~1ulp) ----------
        def trig(x):  # x: [1, n] f32 tile -> (sin, cos) tiles
            n = x.shape[1]
            t = spool.tile([1, n], f32, tag="trig_t")
            q = spool.tile([1, n], f32, tag="trig_q")
            TS(out=t[:], in0=x[:], scalar1=float(2.0 / np.pi), scalar2=None, op0=Alu.mult)
            TS(out=q[:], in0=t[:], scalar1=MAGIC, scalar2=MAGIC, op0=Alu.add, op1=Alu.subtract)
            PIO2_HI = 1.57079601287841796875
            PIO2_LO = float(np.pi / 2 - PIO2_HI)
            r = spool.tile([1, n], f32, tag="trig_r")
            STT(out=r[:], in0=q[:], scalar=-PIO2_HI, in1=x[:], op0=Alu.mult, op1=Alu.add)
            STT(out=r[:], in0=q[:], scalar=-PIO2_LO, in1=r[:], op0=Alu.mult, op1=Alu.add)
            r2 = spool.tile([1, n], f32, tag="trig_r2")
            TT(out=r2[:], in0=r[:], in1=r[:], op=Alu.mult)
            # sin poly
            S = [-1.6666667163e-01, 8.3333337680e-03, -1.9841270114e-04,
                 2.7557314297e-06, -2.5050759689e-08]
            p = spool.tile([1, n], f32, tag="trig_p")
            TS(out=p[:], in0=r2[:], scalar1=S[4], scalar2=S[3], op0=Alu.mult, op1=Alu.add)
            for cf in (S[2], S[1], S[0]):
                TT(out=p[:], in0=p[:], in1=r2[:], op=Alu.mult)
                TS(out=p[:], in0=p[:], scalar1=cf, scalar2=None, op0=Alu.add)
            r3 = spool.tile([1, n], f32, tag="trig_r3")
            TT(out=r3[:], in0=r2[:], in1=r[:], op=Alu.mult)
            sp = spool.tile([1, n], f32, tag="trig_sp")
            TT(out=sp[:], in0=p[:], in1=r3[:], op=Alu.mult)
            TT(out=sp[:], in0=sp[:], in1=r[:], op=Alu.add)
            # cos poly
            C = [4.1666667908e-02, -1.3888889225e-03, 2.4801587642e-05,
                 -2.7557314297e-07]
            cpl = spool.tile([1, n], f32, tag="trig_cp")
            TS(out=cpl[:], in0=r2[:], scalar1=C[3], scalar2=C[2], op0=Alu.mult, op1=Alu.add)
            for cf in (C[1], C[0]):
                TT(out=cpl[:], in0=cpl[:], in1=r2[:], op=Alu.mult)
                TS(out=cpl[:], in0=cpl[:], scalar1=cf, scalar2=None, op0=Alu.add)
            TT(out=cpl[:], in0=cpl[:], in1=r2[:], op=Alu.mult)
            TS(out=cpl[:], in0=cpl[:], scalar1=-0.5, scalar2=None, op0=Alu.add)
            TT(out=cpl[:], in0=cpl[:], in1=r2[:], op=Alu.mult)
            cp = spool.tile([1, n], f32, tag="trig_cpf")
            TS(out=cp[:], in0=cpl[:], scalar1=1.0, scalar2=None, op0=Alu.add)
            # quadrant select: qm = q + 4*(q<0) ; masks
            neg = spool.tile([1, n], f32, tag="trig_neg")
            TS(out=neg[:], in0=q[:], scalar1=0.0, scalar2=None, op0=Alu.is_lt)
            qm = spool.tile([1, n], f32, tag="trig_qm")
            STT(out=qm[:], in0=neg[:], scalar=4.0, in1=q[:], op0=Alu.mult, op1=Alu.add)
            sres = spool.tile([1, n], f32, tag="trig_sres")
            cres = spool.tile([1, n], f32, tag="trig_cres")
            m = spool.tile([1, n], f32, tag="trig_m")
            tm = spool.tile([1, n], f32, tag="trig_tm")
            # sin = m0*sp + m1*cp - m2*sp - m3*cp ; cos = m0*cp - m1*sp - m2*cp + m3*sp
            TS(out=m[:], in0=qm[:], scalar1=0.0, scalar2=None, op0=Alu.is_equal)
            TT(out=sres[:], in0=m[:], in1=sp[:], op=Alu.mult)
            TT(out=cres[:], in0=m[:], in1=cp[:], op=Alu.mult)
            TS(out=m[:], in0=qm[:], scalar1=1.0, scalar2=None, op0=Alu.is_equal)
            TT(out=tm[:], in0=m[:], in1=cp[:], op=Alu.mult)
            TT(out=sres[:], in0=sres[:], in1=tm[:], op=Alu.add)
            TT(out=tm[:], in0=m[:], in1=sp[:], op=Alu.mult)
            TT(out=cres[:], in0=cres[:], in1=tm[:], op=Alu.subtract)
            TS(out=m[:], in0=qm[:], scalar1=2.0, scalar2=None, op0=Alu.is_equal)
            TT(out=tm[:], in0=m[:], in1=sp[:], op=Alu.mult)
            TT(out=sres[:], in0=sres[:], in1=tm[:], op=Alu.subtract)
            TT(out=tm[:], in0=m[:], in1=cp[:], op=Alu.mult)
            TT(out=cres[:], in0=cres[:], in1=tm[:], op=Alu.subtract)
            TS(out=m[:], in0=qm[:], scalar1=3.0, scalar2=None, op0=Alu.is_equal)
            TT(out=tm[:], in0=m[:], in1=cp[:], op=Alu.mult)
            TT(out=sres[:], in0=sres[:], in1=tm[:], op=Alu.subtract)
            TT(out=tm[:], in0=m[:], in1=sp[:], op=Alu.mult)
            TT(out=cres[:], in0=cres[:], in1=tm[:], op=Alu.add)
            return sres, cres

        sinv, cosv = trig(rowsf)   # [1, 6]: cols = (img, angle)

        # ---- SIREN in [LAT, B_LOC] layout -------------------------------
        def sin_reduced(dst, z_sb, bias_col, scale):
            """dst = sin(scale*z + bias) with range reduction; all [LAT, B_LOC]."""
            t = spool.tile([LAT, B_LOC], f32, tag="sir_t")
            if bias_col is not None:
                TS(out=t[:], in0=z_sb[:], scalar1=bias_col, scalar2=float(scale),
                   op0=Alu.add, op1=Alu.mult)
            else:
                TS(out=t[:], in0=z_sb[:], scalar1=float(scale), scalar2=None, op0=Alu.mult)
            u = spool.tile([LAT, B_LOC], f32, tag="sir_u")
            TS(out=u[:], in0=t[:], scalar1=float(1.0 / (2 * np.pi)), scalar2=None, op0=Alu.mult)
            k = spool.tile([LAT, B_LOC], f32, tag="sir_k")
            TS(out=k[:], in0=u[:], scalar1=MAGIC, scalar2=MAGIC, op0=Alu.add, op1=Alu.subtract)
            r = spool.tile([LAT, B_LOC], f32, tag="sir_r")
            STT(out=r[:], in0=k[:], scalar=float(-2 * np.pi), in1=t[:], op0=Alu.mult, op1=Alu.add)
            nc.scalar.activation(out=dst[:], in_=r[:], func=Act.Sin)

        h = spool.tile([LAT, B_LOC], f32, tag="h")
        zp = sppool.tile([LAT, B_LOC], f32, tag="sp")
        nc.tensor.matmul(out=zp[:], lhsT=wm[:, 0:LAT], rhs=latT[:], start=True, stop=True)
        z_sb = spool.tile([LAT, B_LOC], f32, tag="z_sb")
        nc.scalar.copy(out=z_sb[:], in_=zp[:])
        sin_reduced(h, z_sb, bv[:, 0:1], W0_FIRST)
        for li in range(1, 4):
            zp2 = sppool.tile([LAT, B_LOC], f32, tag="sp")
            nc.tensor.matmul(out=zp2[:], lhsT=wm[:, li * LAT:(li + 1) * LAT],
                             rhs=h[:], start=True, stop=True)
            nc.scalar.copy(out=z_sb[:], in_=zp2[:])
            sn = spool.tile([LAT, B_LOC], f32, tag="sir_sn")
            sin_reduced(sn, z_sb, bv[:, li:li + 1], 1.0)
            TT(out=h[:], in0=h[:], in1=sn[:], op=Alu.add)

        # ---- assemble scalar row s [1, 16*B_LOC] ------------------------
        NSC = 16
        s = spool.tile([1, NSC * B_LOC], f32, tag="s_row")

        def ang(b_, k_):  # AP helpers into sinv/cosv columns
            return (3 * b_ + k_, 3 * b_ + k_ + 1)

        t1 = spool.tile([1, 1], f32, tag="t1")
        t2 = spool.tile([1, 1], f32, tag="t2")
        for b_ in range(B_LOC):
            o = NSC * b_
            ca = cosv[:, 3 * b_:3 * b_ + 1]; sa = sinv[:, 3 * b_:3 * b_ + 1]
            cb = cosv[:, 3 * b_ + 1:3 * b_ + 2]; sb = sinv[:, 3 * b_ + 1:3 * b_ + 2]
            cg = cosv[:, 3 * b_ + 2:3 * b_ + 3]; sg = sinv[:, 3 * b_ + 2:3 * b_ + 3]
            # R00 = cg*cb*ca - sg*sa
            TT(out=t1[:], in0=cg, in1=cb, op=Alu.mult)
            TT(out=t1[:], in0=t1[:], in1=ca, op=Alu.mult)
            TT(out=t2[:], in0=sg, in1=sa, op=Alu.mult)
            TT(out=s[:, o + 0:o + 1], in0=t1[:], in1=t2[:], op=Alu.subtract)
            # R01 = cg*cb*sa + sg*ca
            TT(out=t1[:], in0=cg, in1=cb, op=Alu.mult)
            TT(out=t1[:], in0=t1[:], in1=sa, op=Alu.mult)
            TT(out=t2[:], in0=sg, in1=ca, op=Alu.mult)
            TT(out=s[:, o + 1:o + 2], in0=t1[:], in1=t2[:], op=Alu.add)
            # R02 = -cg*sb
            TT(out=t1[:], in0=cg, in1=sb, op=Alu.mult)
            TS(out=s[:, o + 2:o + 3], in0=t1[:], scalar1=-1.0, scalar2=None, op0=Alu.mult)
            # sx + 128
            TS(out=s[:, o + 3:o + 4], in0=shf[:, 2 * b_:2 * b_ + 1],
               scalar1=float(XS // 2), scalar2=None, op0=Alu.add)
            # R10 = -(sg*cb*ca + cg*sa)
            TT(out=t1[:], in0=sg, in1=cb, op=Alu.mult)
            TT(out=t1[:], in0=t1[:], in1=ca, op=Alu.mult)
            TT(out=t2[:], in0=cg, in1=sa, op=Alu.mult)
            TT(out=t1[:], in0=t1[:], in1=t2[:], op=Alu.add)
            TS(out=s[:, o + 4:o + 5], in0=t1[:], scalar1=-1.0, scalar2=None, op0=Alu.mult)
            # R11 = cg*ca - sg*cb*sa
            TT(out=t1[:], in0=sg, in1=cb, op=Alu.mult)
            TT(out=t1[:], in0=t1[:], in1=sa, op=Alu.mult)
            TT(out=t2[:], in0=cg, in1=ca, op=Alu.mult)
            TT(out=s[:, o + 5:o + 6], in0=t2[:], in1=t1[:], op=Alu.subtract)
            # R12 = sg*sb
            TT(out=s[:, o + 6:o + 7], in0=sg, in1=sb, op=Alu.mult)
            # sy + 128
            TS(out=s[:, o + 7:o + 8], in0=shf[:, 2 * b_ + 1:2 * b_ + 2],
               scalar1=float(XS // 2), scalar2=None, op0=Alu.add)
            # h columns via transpose of h[:, b] -> [1, LAT]
            hp = sppool.tile([1, LAT], f32, tag="sp")
            nc.tensor.transpose(out=hp[:], in_=h[:, b_:b_ + 1], identity=idn[:LAT, :LAT])
            nc.vector.tensor_copy(out=s[:, o + 8:o + 16], in_=hp[:])

        # broadcast s across partitions
        bps = sppool.tile([P, NSC * B_LOC], f32, tag="sp")
        nc.tensor.matmul(out=bps[:], lhsT=onesr[:], rhs=s[:], start=True, stop=True)
        bc = spool.tile([P, NSC * B_LOC], f32, tag="bcast")
        nc.vector.tensor_copy(out=bc[:], in_=bps[:])
        if debug:
            nc.sync.dma_start(out=dbg_bc[:], in_=bc[:])

        def bcol(b_, k_):
            return bc[:, NSC * b_ + k_: NSC * b_ + k_ + 1]

        # ---- scatter accumulators ---------------------------------------
        img_ps = [ppool.tile([P, 512], f32, name=f"img_ps{_b}", tag="img_ps") for _b in range(B_LOC)]
        n_chunks_total = cols

        # ---- main loop ---------------------------------------------------
        done_chunks = 0
        for st in range(n_st):
            w = min(st_cols, cols - st * st_cols)
            inp = inpool.tile([P, 13, st_cols], f32, tag="inp")
            nc.sync.dma_start(out=inp[:, :, :w], in_=packed[:, :, st * st_cols: st * st_cols + w])
            cx = inp[:, 0, :w]; cy = inp[:, 1, :w]; cz = inp[:, 2, :w]
            vals_p = inp[:, 3, :w]; bd_p = inp[:, 4, :w]

            vb = plpool.tile([P, st_cols], f32, tag="vb")
            TT(out=vb[:, :w], in0=vals_p, in1=bd_p, op=Alu.add)

            lo16 = []; hi16 = []; v16 = []; nlo16 = []
            for b_ in range(B_LOC):
                # px = cx*R00 + cy*R01 + cz*R02 + (sx+128)
                px = plpool.tile([P, st_cols], f32, tag="px")
                TS(out=px[:, :w], in0=cx, scalar1=bcol(b_, 0), scalar2=None, op0=Alu.mult)
                STT(out=px[:, :w], in0=cy, scalar=bcol(b_, 1), in1=px[:, :w], op0=Alu.mult, op1=Alu.add)
                STT(out=px[:, :w], in0=cz, scalar=bcol(b_, 2), in1=px[:, :w], op0=Alu.mult, op1=Alu.add)
                TS(out=px[:, :w], in0=px[:, :w], scalar1=bcol(b_, 3), scalar2=None, op0=Alu.add)
                py = plpool.tile([P, st_cols], f32, tag="py")
                TS(out=py[:, :w], in0=cx, scalar1=bcol(b_, 4), scalar2=None, op0=Alu.mult)
                STT(out=py[:, :w], in0=cy, scalar=bcol(b_, 5), in1=py[:, :w], op0=Alu.mult, op1=Alu.add)
                STT(out=py[:, :w], in0=cz, scalar=bcol(b_, 6), in1=py[:, :w], op0=Alu.mult, op1=Alu.add)
                TS(out=py[:, :w], in0=py[:, :w], scalar1=bcol(b_, 7), scalar2=None, op0=Alu.add)
                # round + clip
                TS(out=px[:, :w], in0=px[:, :w], scalar1=MAGIC, scalar2=MAGIC, op0=Alu.add, op1=Alu.subtract)
                TS(out=px[:, :w], in0=px[:, :w], scalar1=0.0, scalar2=255.0, op0=Alu.max, op1=Alu.min)
                TS(out=py[:, :w], in0=py[:, :w], scalar1=MAGIC, scalar2=MAGIC, op0=Alu.add, op1=Alu.subtract)
                TS(out=py[:, :w], in0=py[:, :w], scalar1=0.0, scalar2=255.0, op0=Alu.max, op1=Alu.min)
                # hi = floor(py/2) = round(py*0.5 - 0.25) ; m = py - 2*hi ; lo = m*256 + px
                hi = plpool.tile([P, st_cols], f32, tag="hi")
                TS(out=hi[:, :w], in0=py[:, :w], scalar1=0.5, scalar2=-0.25, op0=Alu.mult, op1=Alu.add)
                TS(out=hi[:, :w], in0=hi[:, :w], scalar1=MAGIC, scalar2=MAGIC, op0=Alu.add, op1=Alu.subtract)
                m = plpool.tile([P, st_cols], f32, tag="m")
                STT(out=m[:, :w], in0=hi[:, :w], scalar=-2.0, in1=py[:, :w], op0=Alu.mult, op1=Alu.add)
                lo_t = ohpool.tile([P, st_cols], f32, tag="lo16")
                STT(out=lo_t[:, :w], in0=m[:, :w], scalar=256.0, in1=px[:, :w], op0=Alu.mult, op1=Alu.add)
                nlo_t = ohpool.tile([P, st_cols], f32, tag="nlo16")
                TS(out=nlo_t[:, :w], in0=lo_t[:, :w], scalar1=-1.0, scalar2=None, op0=Alu.mult)
                hi_t = ohpool.tile([P, st_cols], f32, tag="hi16")
                nc.vector.tensor_copy(out=hi_t[:, :w], in_=hi[:, :w])
                # v = vb + sum_l h[l]*Wd_l
                acc = plpool.tile([P, st_cols], f32, tag="acc")
                STT(out=acc[:, :w], in0=inp[:, 5, :w], scalar=bcol(b_, 8), in1=vb[:, :w],
                    op0=Alu.mult, op1=Alu.add)
                for l_ in range(1, LAT):
                    STT(out=acc[:, :w], in0=inp[:, 5 + l_, :w], scalar=bcol(b_, 8 + l_),
                        in1=acc[:, :w], op0=Alu.mult, op1=Alu.add)
                v_t = ohpool.tile([P, st_cols], f32, tag="v16")
                nc.vector.tensor_copy(out=v_t[:, :w], in_=acc[:, :w])
                lo16.append(lo_t); hi16.append(hi_t); v16.append(v_t)
                nlo16.append(nlo_t)
                if debug and st == 0 and b_ == 0:
                    dw = min(64, w)
                    nc.sync.dma_start(out=dbg_pl[0, :, :dw], in_=px[:, :dw])
                    nc.sync.dma_start(out=dbg_pl[1, :, :dw], in_=py[:, :dw])
                    nc.sync.dma_start(out=dbg_pl[2, :, :dw], in_=lo_t[:, :dw])
                    nc.sync.dma_start(out=dbg_pl[3, :, :dw], in_=hi_t[:, :dw])
                    nc.sync.dma_start(out=dbg_pl[4, :, :dw], in_=v_t[:, :dw])

            for c in range(w):
                first = (done_chunks == 0)
                last = (done_chunks == n_chunks_total - 1)
                # ACT engine builds the wide one-hot for a fraction of chunks
                # (relu(1 - |iota - lo|), exact at integers) to offload DVE
                on_act = (done_chunks % ACT_MOD) in ACT_RES
                for b_ in range(B_LOC):
                    oh5 = ohpool.tile([P, 512], f16, tag="oh5", bufs=8)
                    if on_act:
                        ab = ohpool.tile([P, 512], f16, tag="abs_t", bufs=4)
                        nc.scalar.activation(out=ab[:], in_=io512[:], func=Act.Abs,
                                             bias=nlo16[b_][:, c:c + 1])
                        nc.scalar.activation(out=oh5[:], in_=ab[:], func=Act.Relu,
                                             bias=1.0, scale=-1.0)
                    else:
                        TS(out=oh5[:], in0=io512[:], scalar1=lo16[b_][:, c:c + 1],
                           scalar2=None, op0=Alu.is_equal)
                    # v rides the small one-hot: lhsT = onehot(hi) * v
                    oh1 = ohpool.tile([P, P], f16, tag="oh1", bufs=8)
                    TS(out=oh1[:], in0=io128[:], scalar1=hi16[b_][:, c:c + 1],
                       scalar2=v16[b_][:, c:c + 1], op0=Alu.is_equal, op1=Alu.mult)
                    nc.tensor.matmul(out=img_ps[b_][:], lhsT=oh1[:], rhs=oh5[:],
                                     start=first, stop=last, skip_group_check=True)
                done_chunks += 1

        # ---- post-processing per image ----------------------------------
        for b_ in range(B_LOC):
            img_sb = popool.tile([P, 512], f32, tag="img_sb")
            nc.scalar.copy(out=img_sb[:], in_=img_ps[b_][:])
            if debug:
                nc.sync.dma_start(out=dbg_img[b_], in_=img_sb[:])
            # Y-pass: Ty[c][kh] = sum_j A_c(j, kh)^T @ img[:, j*256:...]
            Ty = []
            for ci in range(2):           # 0: real, 1: imag
                tysb = popool.tile([P, 512], f32, tag=f"ty{ci}")
                for kh in range(2):
                    tp = pppool.tile([P, 256], f32, tag="pp")
                    for j in range(2):
                        nc.tensor.matmul(out=tp[:], lhsT=pmat(ci, j, kh),
                                         rhs=img_sb[:, j * 256:(j + 1) * 256],
                                         start=(j == 0), stop=(j == 1))
                    nc.scalar.copy(out=tysb[:, kh * 256:(kh + 1) * 256], in_=tp[:])
                Ty.append(tysb)
            # transpose Ty -> TyT [x-part, ky-free]
            TyT = []
            for ci in range(2):
                ttsb = popool.tile([P, 512], f32, tag=f"tyt{ci}")
                for kh in range(2):
                    for xh in range(2):
                        tp = pppool.tile([P, P], f32, tag="pp")
                        nc.tensor.transpose(
                            out=tp[:], in_=Ty[ci][:, kh * 256 + xh * 128: kh * 256 + (xh + 1) * 128],
                            identity=idn[:])
                        nc.scalar.copy(
                            out=ttsb[:, xh * 256 + kh * 128: xh * 256 + (kh + 1) * 128], in_=tp[:])
                TyT.append(ttsb)
            # X-pass: F[oc][kxh] ; Fr = Cr@Tr - Ci@Ti ; Fi = Cr@Ti + Ci@Tr
            # then G = F * ctf
            G = []
            for oc in range(2):
                gsb = popool.tile([P, 512], f32, tag=f"g{oc}")
                terms = ([(2, 0), (4, 1)] if oc == 0 else [(2, 1), (3, 0)])
                for kxh in range(2):
                    fp = pppool.tile([P, 256], f32, tag="pp")
                    mm = 0
                    for (mk, src) in terms:
                        for xh in range(2):
                            nc.tensor.matmul(out=fp[:], lhsT=pmat(mk, xh, kxh),
                                             rhs=TyT[src][:, xh * 256:(xh + 1) * 256],
                                             start=(mm == 0), stop=(mm == 3))
                            mm += 1
                    TT(out=gsb[:, kxh * 256:(kxh + 1) * 256], in0=fp[:],
                       in1=ctf_sb[:, b_ * 512 + kxh * 256: b_ * 512 + (kxh + 1) * 256],
                       op=Alu.mult)
                G.append(gsb)
            # iX-pass: Z[oc][xh] ; Zr = Vr@Gr - Vi@Gi ; Zi = Vr@Gi + Vi@Gr
            Z = []
            for oc in range(2):
                zsb = popool.tile([P, 512], f32, tag=f"z{oc}")
                terms = ([(5, 0), (7, 1)] if oc == 0 else [(5, 1), (6, 0)])
                for xh in range(2):
                    zp_ = pppool.tile([P, 256], f32, tag="pp")
                    mm = 0
                    for (mk, src) in terms:
                        for kxh in range(2):
                            nc.tensor.matmul(out=zp_[:], lhsT=pmat(mk, kxh, xh),
                                             rhs=G[src][:, kxh * 256:(kxh + 1) * 256],
                                             start=(mm == 0), stop=(mm == 3))
                            mm += 1
                    nc.scalar.copy(out=zsb[:, xh * 256:(xh + 1) * 256], in_=zp_[:])
                Z.append(zsb)
            # transpose Z -> Zt[oc][kyh] [ky-part, x-free]
            Zt = [[], []]
            for oc in range(2):
                for kyh in range(2):
                    ztsb = popool.tile([P, 256], f32, tag=f"zt{oc}{kyh}")
                    for xh in range(2):
                        tp = pppool.tile([P, P], f32, tag="pp")
                        nc.tensor.transpose(
                            out=tp[:], in_=Z[oc][:, xh * 256 + kyh * 128: xh * 256 + (kyh + 1) * 128],
                            identity=idn[:])
                        nc.scalar.copy(out=ztsb[:, xh * 128:(xh + 1) * 128], in_=tp[:])
                    Zt[oc].append(ztsb)
            # iY-pass: out[yh] = sum_kyh (Wy_r@Ztr - Wy_i@Zti)
            for yh in range(2):
                op_ = pppool.tile([P, 256], f32, tag="pp")
                mm = 0
                for (mk, oc) in ((8, 0), (9, 1)):
                    for kyh in range(2):
                        nc.tensor.matmul(out=op_[:], lhsT=pmat(mk, kyh, yh),
                                         rhs=Zt[oc][kyh][:], start=(mm == 0), stop=(mm == 3))
                        mm += 1
                osb = popool.tile([P, 256], f32, tag="osb")
                nc.scalar.copy(out=osb[:], in_=op_[:])
                nc.sync.dma_start(out=out[b_, yh * P:(yh + 1) * P, :], in_=osb[:])

    _split_sync_waits(nc)
    return nc


# ---------------------------------------------------------------------------
# host wrapper
# ---------------------------------------------------------------------------

_PROG_CACHE = {}


def _get_program(cols, debug=False):
    key = (cols, debug)
    if key not in _PROG_CACHE:
        _PROG_CACHE[key] = build_program(cols=cols, debug=debug)
    return _PROG_CACHE[key]


def _marshal(inputs, cols=COLS):
    npad = P * cols
    n_use = min(N, npad)

    def plane(a):
        f = np.zeros(npad, np.float32)
        f[:n_use] = np.asarray(a, np.float32).ravel()[:n_use]
        return f.reshape(P, cols)

    coords = np.asarray(inputs["coords"], np.float32)
    packed = np.zeros((P, 13, cols), np.float32)
    packed[:, 0] = plane(coords[:n_use, 0])
    packed[:, 1] = plane(coords[:n_use, 1])
    packed[:, 2] = plane(coords[:n_use, 2])
    packed[:, 3] = plane(inputs["values"][:n_use])
    packed[:, 4] = plane(inputs["bd"][:n_use])
    Wd = np.asarray(inputs["Wd"], np.float32)
    for l_ in range(LAT):
        packed[:, 5 + l_] = plane(Wd[l_, :n_use])

    iota512 = np.tile(np.arange(512, dtype=np.float16), (P, 1))
    iota128 = np.tile(np.arange(P, dtype=np.float16), (P, 1))
    ones_row = np.ones((1, P), np.float32)
    ident = np.eye(P, dtype=np.float32)
    postmats = np.ascontiguousarray(_post_matrices().transpose(1, 0, 2))  # [128, 10, 512]
    wmats = np.concatenate([np.asarray(inputs[k], np.float32) for k in ("W0", "W1", "W2", "W3")], axis=1)
    bvecs = np.stack([np.asarray(inputs[k], np.float32) for k in ("b0", "b1", "b2", "b3")], axis=1)

    rows = np.asarray(inputs["rows"], np.float32)
    shifts = np.asarray(inputs["shifts"], np.float32)
    latent = np.asarray(inputs["latent"], np.float32)
    ctf = np.asarray(inputs["ctf"], np.float32)

    in_maps = []
    for core in range(N_CORES):
        bs = slice(core * B_LOC, (core + 1) * B_LOC)
        ctf_core = np.concatenate(
            [_ctf_extend_T(c).reshape(P, 512) for c in ctf[bs]], axis=1)
        in_maps.append({
            "packed": packed,
            "rows_flat": rows[bs].reshape(1, 6),
            "shifts_flat": shifts[bs].reshape(1, 4),
            "latentT": np.ascontiguousarray(latent[bs].T),
            "wmats": wmats,
            "bvecs": np.ascontiguousarray(bvecs),
            "ctf_arr": ctf_core,
            "iota512": iota512,
            "iota128": iota128,
            "ones_row": ones_row,
            "ident": ident,
            "postmats": postmats,
        })
    return in_maps


def run(inputs, cols=COLS, trace=False, debug=False):
    from concourse.bass_utils import run_bass_kernel_spmd
    nc = _get_program(cols, debug)
    in_maps = _marshal(inputs, cols)
    res = run_bass_kernel_spmd(nc, in_maps, list(range(N_CORES)), trace=trace)
    outs = [res.results[i]["out"] for i in range(N_CORES)]
    full = np.concatenate(outs, axis=0).astype(np.float32)
    return full, res


def kernel(**inputs):
    out, _ = run(inputs)
    return out



# revision 9
# speedup vs baseline: 3.9070x; 1.1846x over previous
"""Trainium2 Bass kernel for nn_Decoder (scatter_memory).

Strategy: data-parallel over the batch dim (16 images / 8 cores = 2 per core).
Per core:
  - rotation matrices + SIREN hypernet computed on device (poly trig for accuracy)
  - per-point projections px/py and values via DVE pointwise ops over
    point-major [128, cols] planes
  - scatter-add via one-hot matmuls on the TensorEngine: per 128-point chunk,
    lhsT = onehot(hi=flat>>9) [128,128] fp16, rhs = onehot(lo=flat&511)*v
    [128,512] fp16, accumulated into a PSUM bank [128,512] = the 256x256 image
  - gaussian blur + rfft2 * ctf + irfft2 as matmuls against precomputed
    constant (blur-folded) DFT matrices
"""

import os
import sys

import numpy as np

_REPO = "/opt/trn_rl_repo"
if _REPO not in sys.path:
    sys.path.insert(0, _REPO)

B, LAT, N, XS = 16, 8, 500000, 256
W0_FIRST = 30.0
P = 128
N_CORES = 8
B_LOC = B // N_CORES           # images per core
COLS = -(-N // P)              # 3907 point columns per partition
NPAD = P * COLS                # 500096
ST_COLS = 512                  # supertile width (point columns)
MAGIC = 12582912.0             # 1.5 * 2**23 : float32 round-to-nearest-even
ACT_MOD = 9                    # ACT engine takes len(ACT_RES)/ACT_MOD of the
ACT_RES = (4,)                 # wide one-hots (offloads the DVE bottleneck;
                               # measured ACT pass ~1.1us so only a small share pays)

# ---------------------------------------------------------------------------
# host-side constants
# ---------------------------------------------------------------------------

def _gauss_kernel():
    x = np.arange(-3, 4, dtype=np.float64)
    k = np.exp(-0.5 * x * x)
    return k / k.sum()


def _post_matrices():
    """Blur-folded DFT matrices, arranged as matmul lhsT tiles.

    Returns [10, 128, 512] float32: A_r A_i C_r C_i C_mi V_r V_i V_mi Wy_r Wy_mi
    """
    k = _gauss_kernel()
    i = np.arange(XS)
    off = i[:, None] - i[None, :]
    Kb = np.where(np.abs(off) <= 3, k[np.clip(off + 3, 0, 6)], 0.0)
    ang = i[:, None] * i[None, :] * (-2j * np.pi / XS)
    W = np.exp(ang)            # [k, y]
    A = W @ Kb                 # fwd transform with blur folded [k, y]
    Winv = np.exp(-ang)        # [k, y] with e^{+2pi i k y}

    def arr_A(M):  # [p, j, kh, kl] = M[kh*128+kl, 2p+j]
        return np.ascontiguousarray(M.T.reshape(P, 2, 2, P).reshape(P, 512))

    def arr_C(M):  # [xp, xh, kh, kl] = M[kh*128+kl, xh*128+xp]
        return np.ascontiguousarray(
            M.T.reshape(2, P, 2, P).transpose(1, 0, 2, 3).reshape(P, 512))

    def arr_V(M):  # [kp, kh, xh, xl] = M[kh*128+kp, xh*128+xl]
        return np.ascontiguousarray(
            M.reshape(2, P, 2, P).transpose(1, 0, 2, 3).reshape(P, 512))

    mats = [
        arr_A(A.real), arr_A(A.imag),
        arr_C(A.real), arr_C(A.imag), arr_C(-A.imag),
        arr_V(Winv.real), arr_V(Winv.imag), arr_V(-Winv.imag),
        arr_V(Winv.real / (XS * XS)), arr_V(-Winv.imag / (XS * XS)),
    ]
    return np.stack(mats).astype(np.float32)


def _ctf_extend_T(ctf_b):
    """[256 ky, 129 kx] -> [128 kxp, 2 kxh, 256 ky] float32 (Hermitian mirror)."""
    ext = np.zeros((XS, XS), np.float32)       # [ky, kx]
    ext[:, :129] = ctf_b
    ky_idx = (-np.arange(XS)) % XS
    for kx in range(129, XS):
        ext[:, kx] = ctf_b[ky_idx, XS - kx]
    t = ext.T                                   # [kx, ky]
    return np.ascontiguousarray(t.reshape(2, P, XS).transpose(1, 0, 2))


# ---------------------------------------------------------------------------
# tile drain workaround: walrus here accepts only 1 sem wait per instruction
# ---------------------------------------------------------------------------

_PATCHED = False

def _patch_tile_drain():
    global _PATCHED
    if _PATCHED:
        return
    _PATCHED = True
    import concourse.tile as tile_mod
    from concourse.vector_clock import ScopedClock
    from concourse import mybir

    def _drain_and_barrier_split(self, tick_clock, wait_clock):
        nc = self.nc
        drain_inst = nc.sync.drain()
        wait_clock.add_sem_waits(
            drain_inst.ins, ScopedClock({None: tick_clock.global_clock}))
        si = drain_inst.ins.sync_info
        if si is not None and si.on_wait and len(si.on_wait) > 1:
            waits = list(si.on_wait)
            si.on_wait = waits[:1]
            for i in range(1, len(waits)):
                extra = nc.sync.drain()
                esi = extra.ins.sync_info
                if esi is None:
                    extra.ins.sync_info = mybir.SyncInfo(
                        on_wait=[waits[i]], on_update=[])
                else:
                    esi.on_wait = [waits[i]]
        nc.all_engine_barrier()
        assert self.sems is not None
        popped = nc._tile_sem_poison_stack.pop()
        assert popped is self._sem_poison
        nc.clear_and_free_semaphores(list(self.sems.allocated().values()))
        nc.all_engine_barrier()

    tile_mod.TileContext._drain_and_barrier = _drain_and_barrier_split


def _split_sync_waits(nc):
    """walrus here allows only one sem wait per instruction; hoist extras
    onto same-engine NOPs inserted immediately before."""
    from concourse import mybir
    for f in nc.m.functions:
        for bb in f.blocks:
            il = bb.instructions
            out_list = []
            changed = False
            for ins in il:
                si = getattr(ins, "sync_info", None)
                if si is not None and si.on_wait and len(si.on_wait) > 1:
                    waits = list(si.on_wait)
                    for w_ in waits[:-1]:
                        nop = mybir.InstNoOp(
                            name=f"wsplit-{nc.next_id()}", engine=ins.engine,
                            ins=[], outs=[],
                            sync_info=mybir.SyncInfo(on_wait=[w_], on_update=[]))
                        try:
                            nc.register_instruction(nop, overwrite=True)
                        except Exception:
                            pass
                        out_list.append(nop)
                    si.on_wait = waits[-1:]
                    changed = True
                out_list.append(ins)
            if changed:
                bb.instructions = out_list


# ---------------------------------------------------------------------------
# device program
# ---------------------------------------------------------------------------

def build_program(cols=COLS, st_cols=ST_COLS, debug=False):
    _patch_tile_drain()
    from concourse import bass, mybir
    from concourse.tile import TileContext
    from contextlib import ExitStack

    f32 = mybir.dt.float32
    f16 = mybir.dt.float16
    Alu = mybir.AluOpType
    Act = mybir.ActivationFunctionType

    nc = bass.Bass("TRN2", target_bir_lowering=False, debug=False,
                   num_devices=N_CORES)

    # ---- dram parameters -------------------------------------------------
    packed = nc.declare_dram_parameter("packed", [P, 13, cols], f32, isOutput=False)
    rows_flat = nc.declare_dram_parameter("rows_flat", [1, 6], f32, isOutput=False)
    shifts_flat = nc.declare_dram_parameter("shifts_flat", [1, 4], f32, isOutput=False)
    latentT = nc.declare_dram_parameter("latentT", [LAT, B_LOC], f32, isOutput=False)
    wmats = nc.declare_dram_parameter("wmats", [LAT, 4 * LAT], f32, isOutput=False)
    bvecs = nc.declare_dram_parameter("bvecs", [LAT, 4], f32, isOutput=False)
    ctf_arr = nc.declare_dram_parameter("ctf_arr", [P, B_LOC * 512], f32, isOutput=False)
    iota512 = nc.declare_dram_parameter("iota512", [P, 512], f16, isOutput=False)
    iota128 = nc.declare_dram_parameter("iota128", [P, P], f16, isOutput=False)
    ones_row = nc.declare_dram_parameter("ones_row", [1, P], f32, isOutput=False)
    ident = nc.declare_dram_parameter("ident", [P, P], f32, isOutput=False)
    postmats = nc.declare_dram_parameter("postmats", [P, 10, 512], f32, isOutput=False)
    out = nc.declare_dram_parameter("out", [B_LOC, XS, XS], f32, isOutput=True)
    if debug:
        dbg_bc = nc.declare_dram_parameter("dbg_bc", [P, 32], f32, isOutput=True)
        dbg_pl = nc.declare_dram_parameter("dbg_pl", [5, P, 64], f32, isOutput=True)
        dbg_img = nc.declare_dram_parameter("dbg_img", [B_LOC, P, 512], f32, isOutput=True)

    n_st = -(-cols // st_cols)

    with TileContext(nc, num_cores=N_CORES) as tc, ExitStack() as ctx:
        cpool = ctx.enter_context(tc.tile_pool(name="const", bufs=1))
        spool = ctx.enter_context(tc.tile_pool(name="scal", bufs=1))
        ppool = ctx.enter_context(tc.tile_pool(name="psum_s", bufs=2, space="PSUM"))
        sppool = ctx.enter_context(tc.tile_pool(name="psum_t", bufs=2, space="PSUM"))
        inpool = ctx.enter_context(tc.tile_pool(name="inp", bufs=2))
        plpool = ctx.enter_context(tc.tile_pool(name="plane", bufs=2))
        ohpool = ctx.enter_context(tc.tile_pool(name="oh", bufs=6))
        popool = ctx.enter_context(tc.tile_pool(name="post", bufs=1))
        pppool = ctx.enter_context(tc.tile_pool(name="psum_p", bufs=3, space="PSUM"))

        # ---- constants to SBUF ------------------------------------------
        io512 = cpool.tile([P, 512], f16)
        nc.sync.dma_start(out=io512[:], in_=iota512[:])
        io128 = cpool.tile([P, P], f16)
        nc.sync.dma_start(out=io128[:], in_=iota128[:])
        onesr = cpool.tile([1, P], f32)
        nc.sync.dma_start(out=onesr[:], in_=ones_row[:])
        idn = cpool.tile([P, P], f32)
        nc.sync.dma_start(out=idn[:], in_=ident[:])
        pm = cpool.tile([P, 10 * 512], f32)
        nc.sync.dma_start(out=pm[:], in_=postmats[:])
        ctf_sb = cpool.tile([P, B_LOC * 512], f32)
        nc.sync.dma_start(out=ctf_sb[:], in_=ctf_arr[:])
        rowsf = spool.tile([1, 6], f32)
        nc.sync.dma_start(out=rowsf[:], in_=rows_flat[:])
        shf = spool.tile([1, 4], f32)
        nc.sync.dma_start(out=shf[:], in_=shifts_flat[:])
        latT = spool.tile([LAT, B_LOC], f32)
        nc.sync.dma_start(out=latT[:], in_=latentT[:])
        wm = spool.tile([LAT, 4 * LAT], f32)
        nc.sync.dma_start(out=wm[:], in_=wmats[:])
        bv = spool.tile([LAT, 4], f32)
        nc.sync.dma_start(out=bv[:], in_=bvecs[:])

        def pmat(k, a, b):
            # lhsT slice of postmats matrix k, sub-block (a, b) [128, 128]
            return pm[:, k * 512 + a * 256 + b * 128: k * 512 + a * 256 + (b + 1) * 128]

        TT = nc.vector.tensor_tensor
        TS = nc.vector.tensor_scalar
        STT = nc.vector.scalar_tensor_tensor

        # ---- trig: sin/cos of the 6 euler angles (poly, ~1ulp) ----------
        def trig(x):  # x: [1, n] f32 tile -> (sin, cos) tiles
            n = x.shape[1]
            t = spool.tile([1, n], f32, tag="trig_t")
            q = spool.tile([1, n], f32, tag="trig_q")
            TS(out=t[:], in0=x[:], scalar1=float(2.0 / np.pi), scalar2=None, op0=Alu.mult)
            TS(out=q[:], in0=t[:], scalar1=MAGIC, scalar2=MAGIC, op0=Alu.add, op1=Alu.subtract)
            PIO2_HI = 1.57079601287841796875
            PIO2_LO = float(np.pi / 2 - PIO2_HI)
            r = spool.tile([1, n], f32, tag="trig_r")
            STT(out=r[:], in0=q[:], scalar=-PIO2_HI, in1=x[:], op0=Alu.mult, op1=Alu.add)
            STT(out=r[:], in0=q[:], scalar=-PIO2_LO, in1=r[:], op0=Alu.mult, op1=Alu.add)
            r2 = spool.tile([1, n], f32, tag="trig_r2")
            TT(out=r2[:], in0=r[:], in1=r[:], op=Alu.mult)
            # sin poly
            S = [-1.6666667163e-01, 8.3333337680e-03, -1.9841270114e-04,
                 2.7557314297e-06, -2.5050759689e-08]
            p = spool.tile([1, n], f32, tag="trig_p")
            TS(out=p[:], in0=r2[:], scalar1=S[4], scalar2=S[3], op0=Alu.mult, op1=Alu.add)
            for cf in (S[2], S[1], S[0]):
                TT(out=p[:], in0=p[:], in1=r2[:], op=Alu.mult)
                TS(out=p[:], in0=p[:], scalar1=cf, scalar2=None, op0=Alu.add)
            r3 = spool.tile([1, n], f32, tag="trig_r3")
            TT(out=r3[:], in0=r2[:], in1=r[:], op=Alu.mult)
            sp = spool.tile([1, n], f32, tag="trig_sp")
            TT(out=sp[:], in0=p[:], in1=r3[:], op=Alu.mult)
            TT(out=sp[:], in0=sp[:], in1=r[:], op=Alu.add)
            # cos poly
            C = [4.1666667908e-02, -1.3888889225e-03, 2.4801587642e-05,
                 -2.7557314297e-07]
            cpl = spool.tile([1, n], f32, tag="trig_cp")
            TS(out=cpl[:], in0=r2[:], scalar1=C[3], scalar2=C[2], op0=Alu.mult, op1=Alu.add)
            for cf in (C[1], C[0]):
                TT(out=cpl[:], in0=cpl[:], in1=r2[:], op=Alu.mult)
                TS(out=cpl[:], in0=cpl[:], scalar1=cf, scalar2=None, op0=Alu.add)
            TT(out=cpl[:], in0=cpl[:], in1=r2[:], op=Alu.mult)
            TS(out=cpl[:], in0=cpl[:], scalar1=-0.5, scalar2=None, op0=Alu.add)
            TT(out=cpl[:], in0=cpl[:], in1=r2[:], op=Alu.mult)
            cp = spool.tile([1, n], f32, tag="trig_cpf")
            TS(out=cp[:], in0=cpl[:], scalar1=1.0, scalar2=None, op0=Alu.add)
            # quadrant select: qm = q + 4*(q<0) ; masks
            neg = spool.tile([1, n], f32, tag="trig_neg")
            TS(out=neg[:], in0=q[:], scalar1=0.0, scalar2=None, op0=Alu.is_lt)
            qm = spool.tile([1, n], f32, tag="trig_qm")
            STT(out=qm[:], in0=neg[:], scalar=4.0, in1=q[:], op0=Alu.mult, op1=Alu.add)
            sres = spool.tile([1, n], f32, tag="trig_sres")
            cres = spool.tile([1, n], f32, tag="trig_cres")
            m = spool.tile([1, n], f32, tag="trig_m")
            tm = spool.tile([1, n], f32, tag="trig_tm")
            # sin = m0*sp + m1*cp - m2*sp - m3*cp ; cos = m0*cp - m1*sp - m2*cp + m3*sp
            TS(out=m[:], in0=qm[:], scalar1=0.0, scalar2=None, op0=Alu.is_equal)
            TT(out=sres[:], in0=m[:], in1=sp[:], op=Alu.mult)
            TT(out=cres[:], in0=m[:], in1=cp[:], op=Alu.mult)
            TS(out=m[:], in0=qm[:], scalar1=1.0, scalar2=None, op0=Alu.is_equal)
            TT(out=tm[:], in0=m[:], in1=cp[:], op=Alu.mult)
            TT(out=sres[:], in0=sres[:], in1=tm[:], op=Alu.add)
            TT(out=tm[:], in0=m[:], in1=sp[:], op=Alu.mult)
            TT(out=cres[:], in0=cres[:], in1=tm[:], op=Alu.subtract)
            TS(out=m[:], in0=qm[:], scalar1=2.0, scalar2=None, op0=Alu.is_equal)
            TT(out=tm[:], in0=m[:], in1=sp[:], op=Alu.mult)
            TT(out=sres[:], in0=sres[:], in1=tm[:], op=Alu.subtract)
            TT(out=tm[:], in0=m[:], in1=cp[:], op=Alu.mult)
            TT(out=cres[:], in0=cres[:], in1=tm[:], op=Alu.subtract)
            TS(out=m[:], in0=qm[:], scalar1=3.0, scalar2=None, op0=Alu.is_equal)
            TT(out=tm[:], in0=m[:], in1=cp[:], op=Alu.mult)
            TT(out=sres[:], in0=sres[:], in1=tm[:], op=Alu.subtract)
            TT(out=tm[:], in0=m[:], in1=sp[:], op=Alu.mult)
            TT(out=cres[:], in0=cres[:], in1=tm[:], op=Alu.add)
            return sres, cres

        sinv, cosv = trig(rowsf)   # [1, 6]: cols = (img, angle)

        # ---- SIREN in [LAT, B_LOC] layout -------------------------------
        def sin_reduced(dst, z_sb, bias_col, scale):
            """dst = sin(scale*z + bias) with range reduction; all [LAT, B_LOC]."""
            t = spool.tile([LAT, B_LOC], f32, tag="sir_t")
            if bias_col is not None:
                TS(out=t[:], in0=z_sb[:], scalar1=bias_col, scalar2=float(scale),
                   op0=Alu.add, op1=Alu.mult)
            else:
                TS(out=t[:], in0=z_sb[:], scalar1=float(scale), scalar2=None, op0=Alu.mult)
            u = spool.tile([LAT, B_LOC], f32, tag="sir_u")
            TS(out=u[:], in0=t[:], scalar1=float(1.0 / (2 * np.pi)), scalar2=None, op0=Alu.mult)
            k = spool.tile([LAT, B_LOC], f32, tag="sir_k")
            TS(out=k[:], in0=u[:], scalar1=MAGIC, scalar2=MAGIC, op0=Alu.add, op1=Alu.subtract)
            r = spool.tile([LAT, B_LOC], f32, tag="sir_r")
            STT(out=r[:], in0=k[:], scalar=float(-2 * np.pi), in1=t[:], op0=Alu.mult, op1=Alu.add)
            nc.scalar.activation(out=dst[:], in_=r[:], func=Act.Sin)

        h = spool.tile([LAT, B_LOC], f32, tag="h")
        zp = sppool.tile([LAT, B_LOC], f32, tag="sp")
        nc.tensor.matmul(out=zp[:], lhsT=wm[:, 0:LAT], rhs=latT[:], start=True, stop=True)
        z_sb = spool.tile([LAT, B_LOC], f32, tag="z_sb")
        nc.scalar.copy(out=z_sb[:], in_=zp[:])
        sin_reduced(h, z_sb, bv[:, 0:1], W0_FIRST)
        for li in range(1, 4):
            zp2 = sppool.tile([LAT, B_LOC], f32, tag="sp")
            nc.tensor.matmul(out=zp2[:], lhsT=wm[:, li * LAT:(li + 1) * LAT],
                             rhs=h[:], start=True, stop=True)
            nc.scalar.copy(out=z_sb[:], in_=zp2[:])
            sn = spool.tile([LAT, B_LOC], f32, tag="sir_sn")
            sin_reduced(sn, z_sb, bv[:, li:li + 1], 1.0)
            TT(out=h[:], in0=h[:], in1=sn[:], op=Alu.add)

        # ---- assemble scalar row s [1, 16*B_LOC] ------------------------
        NSC = 16
        s = spool.tile([1, NSC * B_LOC], f32, tag="s_row")

        def ang(b_, k_):  # AP helpers into sinv/cosv columns
            return (3 * b_ + k_, 3 * b_ + k_ + 1)

        t1 = spool.tile([1, 1], f32, tag="t1")
        t2 = spool.tile([1, 1], f32, tag="t2")
        for b_ in range(B_LOC):
            o = NSC * b_
            ca = cosv[:, 3 * b_:3 * b_ + 1]; sa = sinv[:, 3 * b_:3 * b_ + 1]
            cb = cosv[:, 3 * b_ + 1:3 * b_ + 2]; sb = sinv[:, 3 * b_ + 1:3 * b_ + 2]
            cg = cosv[:, 3 * b_ + 2:3 * b_ + 3]; sg = sinv[:, 3 * b_ + 2:3 * b_ + 3]
            # R00 = cg*cb*ca - sg*sa
            TT(out=t1[:], in0=cg, in1=cb, op=Alu.mult)
            TT(out=t1[:], in0=t1[:], in1=ca, op=Alu.mult)
            TT(out=t2[:], in0=sg, in1=sa, op=Alu.mult)
            TT(out=s[:, o + 0:o + 1], in0=t1[:], in1=t2[:], op=Alu.subtract)
            # R01 = cg*cb*sa + sg*ca
            TT(out=t1[:], in0=cg, in1=cb, op=Alu.mult)
            TT(out=t1[:], in0=t1[:], in1=sa, op=Alu.mult)
            TT(out=t2[:], in0=sg, in1=ca, op=Alu.mult)
            TT(out=s[:, o + 1:o + 2], in0=t1[:], in1=t2[:], op=Alu.add)
            # R02 = -cg*sb
            TT(out=t1[:], in0=cg, in1=sb, op=Alu.mult)
            TS(out=s[:, o + 2:o + 3], in0=t1[:], scalar1=-1.0, scalar2=None, op0=Alu.mult)
            # sx + 128
            TS(out=s[:, o + 3:o + 4], in0=shf[:, 2 * b_:2 * b_ + 1],
               scalar1=float(XS // 2), scalar2=None, op0=Alu.add)
            # R10 = -(sg*cb*ca + cg*sa)
            TT(out=t1[:], in0=sg, in1=cb, op=Alu.mult)
            TT(out=t1[:], in0=t1[:], in1=ca, op=Alu.mult)
            TT(out=t2[:], in0=cg, in1=sa, op=Alu.mult)
            TT(out=t1[:], in0=t1[:], in1=t2[:], op=Alu.add)
            TS(out=s[:, o + 4:o + 5], in0=t1[:], scalar1=-1.0, scalar2=None, op0=Alu.mult)
            # R11 = cg*ca - sg*cb*sa
            TT(out=t1[:], in0=sg, in1=cb, op=Alu.mult)
            TT(out=t1[:], in0=t1[:], in1=sa, op=Alu.mult)
            TT(out=t2[:], in0=cg, in1=ca, op=Alu.mult)
            TT(out=s[:, o + 5:o + 6], in0=t2[:], in1=t1[:], op=Alu.subtract)
            # R12 = sg*sb
            TT(out=s[:, o + 6:o + 7], in0=sg, in1=sb, op=Alu.mult)
            # sy + 128
            TS(out=s[:, o + 7:o + 8], in0=shf[:, 2 * b_ + 1:2 * b_ + 2],
               scalar1=float(XS // 2), scalar2=None, op0=Alu.add)
            # h columns via transpose of h[:, b] -> [1, LAT]
            hp = sppool.tile([1, LAT], f32, tag="sp")
            nc.tensor.transpose(out=hp[:], in_=h[:, b_:b_ + 1], identity=idn[:LAT, :LAT])
            nc.vector.tensor_copy(out=s[:, o + 8:o + 16], in_=hp[:])

        # broadcast s across partitions
        bps = sppool.tile([P, NSC * B_LOC], f32, tag="sp")
        nc.tensor.matmul(out=bps[:], lhsT=onesr[:], rhs=s[:], start=True, stop=True)
        bc = spool.tile([P, NSC * B_LOC], f32, tag="bcast")
        nc.vector.tensor_copy(out=bc[:], in_=bps[:])
        if debug:
            nc.sync.dma_start(out=dbg_bc[:], in_=bc[:])

        def bcol(b_, k_):
            return bc[:, NSC * b_ + k_: NSC * b_ + k_ + 1]

        # ---- scatter accumulators ---------------------------------------
        img_ps = [ppool.tile([P, 512], f32, name=f"img_ps{_b}", tag="img_ps") for _b in range(B_LOC)]
        n_chunks_total = cols

        # ---- main loop ---------------------------------------------------
        done_chunks = 0
        for st in range(n_st):
            w = min(st_cols, cols - st * st_cols)
            inp = inpool.tile([P, 13, st_cols], f32, tag="inp")
            nc.sync.dma_start(out=inp[:, :, :w], in_=packed[:, :, st * st_cols: st * st_cols + w])
            cx = inp[:, 0, :w]; cy = inp[:, 1, :w]; cz = inp[:, 2, :w]
            vals_p = inp[:, 3, :w]; bd_p = inp[:, 4, :w]

            vb = plpool.tile([P, st_cols], f32, tag="vb")
            TT(out=vb[:, :w], in0=vals_p, in1=bd_p, op=Alu.add)

            lo16 = []; hi16 = []; v16 = []; nlo16 = []
            for b_ in range(B_LOC):
                # px = cx*R00 + cy*R01 + cz*R02 + (sx+128)
                px = plpool.tile([P, st_cols], f32, tag="px")
                TS(out=px[:, :w], in0=cx, scalar1=bcol(b_, 0), scalar2=None, op0=Alu.mult)
                STT(out=px[:, :w], in0=cy, scalar=bcol(b_, 1), in1=px[:, :w], op0=Alu.mult, op1=Alu.add)
                STT(out=px[:, :w], in0=cz, scalar=bcol(b_, 2), in1=px[:, :w], op0=Alu.mult, op1=Alu.add)
                TS(out=px[:, :w], in0=px[:, :w], scalar1=bcol(b_, 3), scalar2=None, op0=Alu.add)
                py = plpool.tile([P, st_cols], f32, tag="py")
                TS(out=py[:, :w], in0=cx, scalar1=bcol(b_, 4), scalar2=None, op0=Alu.mult)
                STT(out=py[:, :w], in0=cy, scalar=bcol(b_, 5), in1=py[:, :w], op0=Alu.mult, op1=Alu.add)
                STT(out=py[:, :w], in0=cz, scalar=bcol(b_, 6), in1=py[:, :w], op0=Alu.mult, op1=Alu.add)
                TS(out=py[:, :w], in0=py[:, :w], scalar1=bcol(b_, 7), scalar2=None, op0=Alu.add)
                # round + clip
                TS(out=px[:, :w], in0=px[:, :w], scalar1=MAGIC, scalar2=MAGIC, op0=Alu.add, op1=Alu.subtract)
                TS(out=px[:, :w], in0=px[:, :w], scalar1=0.0, scalar2=255.0, op0=Alu.max, op1=Alu.min)
                TS(out=py[:, :w], in0=py[:, :w], scalar1=MAGIC, scalar2=MAGIC, op0=Alu.add, op1=Alu.subtract)
                TS(out=py[:, :w], in0=py[:, :w], scalar1=0.0, scalar2=255.0, op0=Alu.max, op1=Alu.min)
                # hi = floor(py/2) = round(py*0.5 - 0.25) ; m = py - 2*hi ; lo = m*256 + px
                hi = plpool.tile([P, st_cols], f32, tag="hi")
                TS(out=hi[:, :w], in0=py[:, :w], scalar1=0.5, scalar2=-0.25, op0=Alu.mult, op1=Alu.add)
                TS(out=hi[:, :w], in0=hi[:, :w], scalar1=MAGIC, scalar2=MAGIC, op0=Alu.add, op1=Alu.subtract)
                m = plpool.tile([P, st_cols], f32, tag="m")
                STT(out=m[:, :w], in0=hi[:, :w], scalar=-2.0, in1=py[:, :w], op0=Alu.mult, op1=Alu.add)
                lo_t = ohpool.tile([P, st_cols], f32, tag="lo16")
                STT(out=lo_t[:, :w], in0=m[:, :w], scalar=256.0, in1=px[:, :w], op0=Alu.mult, op1=Alu.add)
                nlo_t = ohpool.tile([P, st_cols], f32, tag="nlo16")
                TS(out=nlo_t[:, :w], in0=lo_t[:, :w], scalar1=-1.0, scalar2=None, op0=Alu.mult)
                hi_t = ohpool.tile([P, st_cols], f32, tag="hi16")
                nc.vector.tensor_copy(out=hi_t[:, :w], in_=hi[:, :w])
                # v = vb + sum_l h[l]*Wd_l
                acc = plpool.tile([P, st_cols], f32, tag="acc")
                STT(out=acc[:, :w], in0=inp[:, 5, :w], scalar=bcol(b_, 8), in1=vb[:, :w],
                    op0=Alu.mult, op1=Alu.add)
                for l_ in range(1, LAT):
                    STT(out=acc[:, :w], in0=inp[:, 5 + l_, :w], scalar=bcol(b_, 8 + l_),
                        in1=acc[:, :w], op0=Alu.mult, op1=Alu.add)
                v_t = ohpool.tile([P, st_cols], f32, tag="v16")
                nc.vector.tensor_copy(out=v_t[:, :w], in_=acc[:, :w])
                lo16.append(lo_t); hi16.append(hi_t); v16.append(v_t)
                nlo16.append(nlo_t)
                if debug and st == 0 and b_ == 0:
                    dw = min(64, w)
                    nc.sync.dma_start(out=dbg_pl[0, :, :dw], in_=px[:, :dw])
                    nc.sync.dma_start(out=dbg_pl[1, :, :dw], in_=py[:, :dw])
                    nc.sync.dma_start(out=dbg_pl[2, :, :dw], in_=lo_t[:, :dw])
                    nc.sync.dma_start(out=dbg_pl[3, :, :dw], in_=hi_t[:, :dw])
                    nc.sync.dma_start(out=dbg_pl[4, :, :dw], in_=v_t[:, :dw])

            for c in range(w):
                first = (done_chunks == 0)
                last = (done_chunks == n_chunks_total - 1)
                # ACT engine builds the wide one-hot for a fraction of chunks
                # (relu(1 - |iota - lo|), exact at integers) to offload DVE
                on_act = (done_chunks % ACT_MOD) in ACT_RES
                for b_ in range(B_LOC):
                    oh5 = ohpool.tile([P, 512], f16, tag="oh5", bufs=8)
                    if on_act:
                        ab = ohpool.tile([P, 512], f16, tag="abs_t", bufs=4)
                        nc.scalar.activation(out=ab[:], in_=io512[:], func=Act.Abs,
                                             bias=nlo16[b_][:, c:c + 1])
                        nc.scalar.activation(out=oh5[:], in_=ab[:], func=Act.Relu,
                                             bias=1.0, scale=-1.0)
                    else:
                        TS(out=oh5[:], in0=io512[:], scalar1=lo16[b_][:, c:c + 1],
                           scalar2=None, op0=Alu.is_equal)
                    # v rides the small one-hot: lhsT = onehot(hi) * v
                    oh1 = ohpool.tile([P, P], f16, tag="oh1", bufs=8)
                    TS(out=oh1[:], in0=io128[:], scalar1=hi16[b_][:, c:c + 1],
                       scalar2=v16[b_][:, c:c + 1], op0=Alu.is_equal, op1=Alu.mult)
                    nc.tensor.matmul(out=img_ps[b_][:], lhsT=oh1[:], rhs=oh5[:],
                                     start=first, stop=last, skip_group_check=True)
                done_chunks += 1

        # ---- post-processing per image ----------------------------------
        for b_ in range(B_LOC):
            img_sb = popool.tile([P, 512], f32, tag="img_sb")
            nc.scalar.copy(out=img_sb[:], in_=img_ps[b_][:])
            if debug:
                nc.sync.dma_start(out=dbg_img[b_], in_=img_sb[:])
            # Y-pass: Ty[c][kh] = sum_j A_c(j, kh)^T @ img[:, j*256:...]
            Ty = []
            for ci in range(2):           # 0: real, 1: imag
                tysb = popool.tile([P, 512], f32, tag=f"ty{ci}")
                for kh in range(2):
                    tp = pppool.tile([P, 256], f32, tag="pp")
                    for j in range(2):
                        nc.tensor.matmul(out=tp[:], lhsT=pmat(ci, j, kh),
                                         rhs=img_sb[:, j * 256:(j + 1) * 256],
                                         start=(j == 0), stop=(j == 1))
                    nc.scalar.copy(out=tysb[:, kh * 256:(kh + 1) * 256], in_=tp[:])
                Ty.append(tysb)
            # transpose Ty -> TyT [x-part, ky-free]
            TyT = []
            for ci in range(2):
                ttsb = popool.tile([P, 512], f32, tag=f"tyt{ci}")
                for kh in range(2):
                    for xh in range(2):
                        tp = pppool.tile([P, P], f32, tag="pp")
                        nc.tensor.transpose(
                            out=tp[:], in_=Ty[ci][:, kh * 256 + xh * 128: kh * 256 + (xh + 1) * 128],
                            identity=idn[:])
                        nc.scalar.copy(
                            out=ttsb[:, xh * 256 + kh * 128: xh * 256 + (kh + 1) * 128], in_=tp[:])
                TyT.append(ttsb)
            # X-pass: F[oc][kxh] ; Fr = Cr@Tr - Ci@Ti ; Fi = Cr@Ti + Ci@Tr
            # then G = F * ctf
            G = []
            for oc in range(2):
                gsb = popool.tile([P, 512], f32, tag=f"g{oc}")
                terms = ([(2, 0), (4, 1)] if oc == 0 else [(2, 1), (3, 0)])
                for kxh in range(2):
                    fp = pppool.tile([P, 256], f32, tag="pp")
                    mm = 0
                    for (mk, src) in terms:
                        for xh in range(2):
                            nc.tensor.matmul(out=fp[:], lhsT=pmat(mk, xh, kxh),
                                             rhs=TyT[src][:, xh * 256:(xh + 1) * 256],
                                             start=(mm == 0), stop=(mm == 3))
                            mm += 1
                    TT(out=gsb[:, kxh * 256:(kxh + 1) * 256], in0=fp[:],
                       in1=ctf_sb[:, b_ * 512 + kxh * 256: b_ * 512 + (kxh + 1) * 256],
                       op=Alu.mult)
                G.append(gsb)
            # iX-pass: Z[oc][xh] ; Zr = Vr@Gr - Vi@Gi ; Zi = Vr@Gi + Vi@Gr
            Z = []
            for oc in range(2):
                zsb = popool.tile([P, 512], f32, tag=f"z{oc}")
                terms = ([(5, 0), (7, 1)] if oc == 0 else [(5, 1), (6, 0)])
                for xh in range(2):
                    zp_ = pppool.tile([P, 256], f32, tag="pp")
                    mm = 0
                    for (mk, src) in terms:
                        for kxh in range(2):
                            nc.tensor.matmul(out=zp_[:], lhsT=pmat(mk, kxh, xh),
                                             rhs=G[src][:, kxh * 256:(kxh + 1) * 256],
                                             start=(mm == 0), stop=(mm == 3))
                            mm += 1
                    nc.scalar.copy(out=zsb[:, xh * 256:(xh + 1) * 256], in_=zp_[:])
                Z.append(zsb)
            # transpose Z -> Zt[oc][kyh] [ky-part, x-free]
            Zt = [[], []]
            for oc in range(2):
                for kyh in range(2):
                    ztsb = popool.tile([P, 256], f32, tag=f"zt{oc}{kyh}")
                    for xh in range(2):
                        tp = pppool.tile([P, P], f32, tag="pp")
                        nc.tensor.transpose(
                            out=tp[:], in_=Z[oc][:, xh * 256 + kyh * 128: xh * 256 + (kyh + 1) * 128],
                            identity=idn[:])
                        nc.scalar.copy(out=ztsb[:, xh * 128:(xh + 1) * 128], in_=tp[:])
                    Zt[oc].append(ztsb)
            # iY-pass: out[yh] = sum_kyh (Wy_r@Ztr - Wy_i@Zti)
            for yh in range(2):
                op_ = pppool.tile([P, 256], f32, tag="pp")
                mm = 0
                for (mk, oc) in ((8, 0), (9, 1)):
                    for kyh in range(2):
                        nc.tensor.matmul(out=op_[:], lhsT=pmat(mk, kyh, yh),
                                         rhs=Zt[oc][kyh][:], start=(mm == 0), stop=(mm == 3))
                        mm += 1
                osb = popool.tile([P, 256], f32, tag="osb")
                nc.scalar.copy(out=osb[:], in_=op_[:])
                nc.sync.dma_start(out=out[b_, yh * P:(yh + 1) * P, :], in_=osb[:])

    _split_sync_waits(nc)
    return nc


# ---------------------------------------------------------------------------
# host wrapper
# ---------------------------------------------------------------------------

_PROG_CACHE = {}


def _get_program(cols, debug=False):
    key = (cols, debug)
    if key not in _PROG_CACHE:
        _PROG_CACHE[key] = build_program(cols=cols, debug=debug)
    return _PROG_CACHE[key]


def _marshal(inputs, cols=COLS):
    npad = P * cols
    n_use = min(N, npad)

    def plane(a):
        f = np.zeros(npad, np.float32)
        f[:n_use] = np.asarray(a, np.float32).ravel()[:n_use]
        return f.reshape(P, cols)

    coords = np.asarray(inputs["coords"], np.float32)
    packed = np.zeros((P, 13, cols), np.float32)
    packed[:, 0] = plane(coords[:n_use, 0])
    packed[:, 1] = plane(coords[:n_use, 1])
    packed[:, 2] = plane(coords[:n_use, 2])
    packed[:, 3] = plane(inputs["values"][:n_use])
    packed[:, 4] = plane(inputs["bd"][:n_use])
    Wd = np.asarray(inputs["Wd"], np.float32)
    for l_ in range(LAT):
        packed[:, 5 + l_] = plane(Wd[l_, :n_use])

    iota512 = np.tile(np.arange(512, dtype=np.float16), (P, 1))
    iota128 = np.tile(np.arange(P, dtype=np.float16), (P, 1))
    ones_row = np.ones((1, P), np.float32)
    ident = np.eye(P, dtype=np.float32)
    postmats = np.ascontiguousarray(_post_matrices().transpose(1, 0, 2))  # [128, 10, 512]
    wmats = np.concatenate([np.asarray(inputs[k], np.float32) for k in ("W0", "W1", "W2", "W3")], axis=1)
    bvecs = np.stack([np.asarray(inputs[k], np.float32) for k in ("b0", "b1", "b2", "b3")], axis=1)

    rows = np.asarray(inputs["rows"], np.float32)
    shifts = np.asarray(inputs["shifts"], np.float32)
    latent = np.asarray(inputs["latent"], np.float32)
    ctf = np.asarray(inputs["ctf"], np.float32)

    in_maps = []
    for core in range(N_CORES):
        bs = slice(core * B_LOC, (core + 1) * B_LOC)
        ctf_core = np.concatenate(
            [_ctf_extend_T(c).reshape(P, 512) for c in ctf[bs]], axis=1)
        in_maps.append({
            "packed": packed,
            "rows_flat": rows[bs].reshape(1, 6),
            "shifts_flat": shifts[bs].reshape(1, 4),
            "latentT": np.ascontiguousarray(latent[bs].T),
            "wmats": wmats,
            "bvecs": np.ascontiguousarray(bvecs),
            "ctf_arr": ctf_core,
            "iota512": iota512,
            "iota128": iota128,
            "ones_row": ones_row,
            "ident": ident,
            "postmats": postmats,
        })
    return in_maps


def run(inputs, cols=COLS, trace=False, debug=False):
    from concourse.bass_utils import run_bass_kernel_spmd
    nc = _get_program(cols, debug)
    in_maps = _marshal(inputs, cols)
    res = run_bass_kernel_spmd(nc, in_maps, list(range(N_CORES)), trace=trace)
    outs = [res.results[i]["out"] for i in range(N_CORES)]
    full = np.concatenate(outs, axis=0).astype(np.float32)
    return full, res


def kernel(**inputs):
    out, _ = run(inputs)
    return out



# revision 17
# speedup vs baseline: 4.1730x; 1.0681x over previous
"""Trainium2 Bass kernel for nn_Decoder (scatter_memory).

Strategy: data-parallel over the batch dim (16 images / 8 cores = 2 per core).
Per core:
  - rotation matrices + SIREN hypernet computed on device (poly trig for accuracy)
  - per-point projections px/py and values via DVE pointwise ops over
    point-major [128, cols] planes
  - scatter-add via one-hot matmuls on the TensorEngine: per 128-point chunk,
    lhsT = onehot(hi=flat>>9) [128,128] fp16, rhs = onehot(lo=flat&511)*v
    [128,512] fp16, accumulated into a PSUM bank [128,512] = the 256x256 image
  - gaussian blur + rfft2 * ctf + irfft2 as matmuls against precomputed
    constant (blur-folded) DFT matrices
"""

import os
import sys

import numpy as np

_REPO = "/opt/trn_rl_repo"
if _REPO not in sys.path:
    sys.path.insert(0, _REPO)

B, LAT, N, XS = 16, 8, 500000, 256
W0_FIRST = 30.0
P = 128
N_CORES = 8
B_LOC = B // N_CORES           # images per core
COLS = -(-N // P)              # 3907 point columns per partition
NPAD = P * COLS                # 500096
ST_COLS = 512                  # supertile width (point columns)
MAGIC = 12582912.0             # 1.5 * 2**23 : float32 round-to-nearest-even
import os as _os
ACT_MOD = 9                    # ACT engine takes len(ACT_RES)/ACT_MOD of the
ACT_RES = tuple(               # wide one-hots (offloads the DVE bottleneck;
    int(x) for x in _os.environ.get("K_ACT_RES", "1,4,7").split(",") if x.strip() != "")
V_IN_OH1 = _os.environ.get("K_V_IN_OH1", "1") == "1"
POOL_RES = tuple(              # chunks whose wide one-hot goes to gpsimd
    int(x) for x in _os.environ.get("K_POOL_RES", "").split(",") if x.strip() != "")
OH_BUFS = int(_os.environ.get("K_OH_BUFS", "16"))

# ---------------------------------------------------------------------------
# host-side constants
# ---------------------------------------------------------------------------

def _gauss_kernel():
    x = np.arange(-3, 4, dtype=np.float64)
    k = np.exp(-0.5 * x * x)
    return k / k.sum()


def _post_matrices():
    """Blur-folded DFT matrices, arranged as matmul lhsT tiles.

    Returns [10, 128, 512] float32: A_r A_i C_r C_i C_mi V_r V_i V_mi Wy_r Wy_mi
    """
    k = _gauss_kernel()
    i = np.arange(XS)
    off = i[:, None] - i[None, :]
    Kb = np.where(np.abs(off) <= 3, k[np.clip(off + 3, 0, 6)], 0.0)
    ang = i[:, None] * i[None, :] * (-2j * np.pi / XS)
    W = np.exp(ang)            # [k, y]
    A = W @ Kb                 # fwd transform with blur folded [k, y]
    Winv = np.exp(-ang)        # [k, y] with e^{+2pi i k y}

    def arr_A(M):  # [p, j, kh, kl] = M[kh*128+kl, 2p+j]
        return np.ascontiguousarray(M.T.reshape(P, 2, 2, P).reshape(P, 512))

    def arr_C(M):  # [xp, xh, kh, kl] = M[kh*128+kl, xh*128+xp]
        return np.ascontiguousarray(
            M.T.reshape(2, P, 2, P).transpose(1, 0, 2, 3).reshape(P, 512))

    def arr_V(M):  # [kp, kh, xh, xl] = M[kh*128+kp, xh*128+xl]
        return np.ascontiguousarray(
            M.reshape(2, P, 2, P).transpose(1, 0, 2, 3).reshape(P, 512))

    mats = [
        arr_A(A.real), arr_A(A.imag),
        arr_C(A.real), arr_C(A.imag), arr_C(-A.imag),
        arr_V(Winv.real), arr_V(Winv.imag), arr_V(-Winv.imag),
        arr_V(Winv.real / (XS * XS)), arr_V(-Winv.imag / (XS * XS)),
    ]
    return np.stack(mats).astype(np.float32)


def _ctf_extend_T(ctf_b):
    """[256 ky, 129 kx] -> [128 kxp, 2 kxh, 256 ky] float32 (Hermitian mirror)."""
    ext = np.zeros((XS, XS), np.float32)       # [ky, kx]
    ext[:, :129] = ctf_b
    ky_idx = (-np.arange(XS)) % XS
    for kx in range(129, XS):
        ext[:, kx] = ctf_b[ky_idx, XS - kx]
    t = ext.T                                   # [kx, ky]
    return np.ascontiguousarray(t.reshape(2, P, XS).transpose(1, 0, 2))


# ---------------------------------------------------------------------------
# tile drain workaround: walrus here accepts only 1 sem wait per instruction
# ---------------------------------------------------------------------------

_PATCHED = False

def _patch_tile_drain():
    global _PATCHED
    if _PATCHED:
        return
    _PATCHED = True
    import concourse.tile as tile_mod
    from concourse.vector_clock import ScopedClock
    from concourse import mybir

    def _drain_and_barrier_split(self, tick_clock, wait_clock):
        nc = self.nc
        drain_inst = nc.sync.drain()
        wait_clock.add_sem_waits(
            drain_inst.ins, ScopedClock({None: tick_clock.global_clock}))
        si = drain_inst.ins.sync_info
        if si is not None and si.on_wait and len(si.on_wait) > 1:
            waits = list(si.on_wait)
            si.on_wait = waits[:1]
            for i in range(1, len(waits)):
                extra = nc.sync.drain()
                esi = extra.ins.sync_info
                if esi is None:
                    extra.ins.sync_info = mybir.SyncInfo(
                        on_wait=[waits[i]], on_update=[])
                else:
                    esi.on_wait = [waits[i]]
        nc.all_engine_barrier()
        assert self.sems is not None
        popped = nc._tile_sem_poison_stack.pop()
        assert popped is self._sem_poison
        nc.clear_and_free_semaphores(list(self.sems.allocated().values()))
        nc.all_engine_barrier()

    tile_mod.TileContext._drain_and_barrier = _drain_and_barrier_split


def _split_sync_waits(nc):
    """walrus here allows only one sem wait per instruction; hoist extras
    onto same-engine NOPs inserted immediately before."""
    from concourse import mybir
    for f in nc.m.functions:
        for bb in f.blocks:
            il = bb.instructions
            out_list = []
            changed = False
            for ins in il:
                si = getattr(ins, "sync_info", None)
                if si is not None and si.on_wait and len(si.on_wait) > 1:
                    waits = list(si.on_wait)
                    for w_ in waits[:-1]:
                        nop = mybir.InstNoOp(
                            name=f"wsplit-{nc.next_id()}", engine=ins.engine,
                            ins=[], outs=[],
                            sync_info=mybir.SyncInfo(on_wait=[w_], on_update=[]))
                        try:
                            nc.register_instruction(nop, overwrite=True)
                        except Exception:
                            pass
                        out_list.append(nop)
                    si.on_wait = waits[-1:]
                    changed = True
                out_list.append(ins)
            if changed:
                bb.instructions = out_list


# ---------------------------------------------------------------------------
# device program
# ---------------------------------------------------------------------------

def build_program(cols=COLS, st_cols=ST_COLS, debug=False):
    _patch_tile_drain()
    from concourse import bass, mybir
    from concourse.tile import TileContext
    from contextlib import ExitStack

    f32 = mybir.dt.float32
    f16 = mybir.dt.float16
    Alu = mybir.AluOpType
    Act = mybir.ActivationFunctionType

    nc = bass.Bass("TRN2", target_bir_lowering=False, debug=False,
                   num_devices=N_CORES)

    # ---- dram parameters -------------------------------------------------
    packed = nc.declare_dram_parameter("packed", [P, 13, cols], f32, isOutput=False)
    rows_flat = nc.declare_dram_parameter("rows_flat", [1, 6], f32, isOutput=False)
    shifts_flat = nc.declare_dram_parameter("shifts_flat", [1, 4], f32, isOutput=False)
    latentT = nc.declare_dram_parameter("latentT", [LAT, B_LOC], f32, isOutput=False)
    wmats = nc.declare_dram_parameter("wmats", [LAT, 4 * LAT], f32, isOutput=False)
    bvecs = nc.declare_dram_parameter("bvecs", [LAT, 4], f32, isOutput=False)
    ctf_arr = nc.declare_dram_parameter("ctf_arr", [P, B_LOC * 512], f32, isOutput=False)
    iota512 = nc.declare_dram_parameter("iota512", [P, 512], f16, isOutput=False)
    iota128 = nc.declare_dram_parameter("iota128", [P, P], f16, isOutput=False)
    ones_row = nc.declare_dram_parameter("ones_row", [1, P], f32, isOutput=False)
    ident = nc.declare_dram_parameter("ident", [P, P], f32, isOutput=False)
    postmats = nc.declare_dram_parameter("postmats", [P, 10, 512], f32, isOutput=False)
    out = nc.declare_dram_parameter("out", [B_LOC, XS, XS], f32, isOutput=True)
    if debug:
        dbg_bc = nc.declare_dram_parameter("dbg_bc", [P, 32], f32, isOutput=True)
        dbg_pl = nc.declare_dram_parameter("dbg_pl", [5, P, 64], f32, isOutput=True)
        dbg_img = nc.declare_dram_parameter("dbg_img", [B_LOC, P, 512], f32, isOutput=True)

    n_st = -(-cols // st_cols)

    with TileContext(nc, num_cores=N_CORES) as tc, ExitStack() as ctx:
        cpool = ctx.enter_context(tc.tile_pool(name="const", bufs=1))
        spool = ctx.enter_context(tc.tile_pool(name="scal", bufs=1))
        ppool = ctx.enter_context(tc.tile_pool(name="psum_s", bufs=2, space="PSUM"))
        sppool = ctx.enter_context(tc.tile_pool(name="psum_t", bufs=2, space="PSUM"))
        inpool = ctx.enter_context(tc.tile_pool(name="inp", bufs=2))
        plpool = ctx.enter_context(tc.tile_pool(name="plane", bufs=2))
        ohpool = ctx.enter_context(tc.tile_pool(name="oh", bufs=6))
        popool = ctx.enter_context(tc.tile_pool(name="post", bufs=1))
        pppool = ctx.enter_context(tc.tile_pool(name="psum_p", bufs=3, space="PSUM"))

        # ---- constants to SBUF ------------------------------------------
        io512 = cpool.tile([P, 512], f16)
        nc.sync.dma_start(out=io512[:], in_=iota512[:])
        io128 = cpool.tile([P, P], f16)
        nc.sync.dma_start(out=io128[:], in_=iota128[:])
        onesr = cpool.tile([1, P], f32)
        nc.sync.dma_start(out=onesr[:], in_=ones_row[:])
        idn = cpool.tile([P, P], f32)
        nc.sync.dma_start(out=idn[:], in_=ident[:])
        pm = cpool.tile([P, 10 * 512], f32)
        nc.sync.dma_start(out=pm[:], in_=postmats[:])
        ctf_sb = cpool.tile([P, B_LOC * 512], f32)
        nc.sync.dma_start(out=ctf_sb[:], in_=ctf_arr[:])
        rowsf = spool.tile([1, 6], f32)
        nc.sync.dma_start(out=rowsf[:], in_=rows_flat[:])
        shf = spool.tile([1, 4], f32)
        nc.sync.dma_start(out=shf[:], in_=shifts_flat[:])
        latT = spool.tile([LAT, B_LOC], f32)
        nc.sync.dma_start(out=latT[:], in_=latentT[:])
        wm = spool.tile([LAT, 4 * LAT], f32)
        nc.sync.dma_start(out=wm[:], in_=wmats[:])
        bv = spool.tile([LAT, 4], f32)
        nc.sync.dma_start(out=bv[:], in_=bvecs[:])

        def pmat(k, a, b):
            # lhsT slice of postmats matrix k, sub-block (a, b) [128, 128]
            return pm[:, k * 512 + a * 256 + b * 128: k * 512 + a * 256 + (b + 1) * 128]

        TT = nc.vector.tensor_tensor
        TS = nc.vector.tensor_scalar
        STT = nc.vector.scalar_tensor_tensor

        # ---- trig: sin/cos of the 6 euler angles (poly, ~1ulp) ----------
        def trig(x):  # x: [1, n] f32 tile -> (sin, cos) tiles
            n = x.shape[1]
            t = spool.tile([1, n], f32, tag="trig_t")
            q = spool.tile([1, n], f32, tag="trig_q")
            TS(out=t[:], in0=x[:], scalar1=float(2.0 / np.pi), scalar2=None, op0=Alu.mult)
            TS(out=q[:], in0=t[:], scalar1=MAGIC, scalar2=MAGIC, op0=Alu.add, op1=Alu.subtract)
            PIO2_HI = 1.57079601287841796875
            PIO2_LO = float(np.pi / 2 - PIO2_HI)
            r = spool.tile([1, n], f32, tag="trig_r")
            STT(out=r[:], in0=q[:], scalar=-PIO2_HI, in1=x[:], op0=Alu.mult, op1=Alu.add)
            STT(out=r[:], in0=q[:], scalar=-PIO2_LO, in1=r[:], op0=Alu.mult, op1=Alu.add)
            r2 = spool.tile([1, n], f32, tag="trig_r2")
            TT(out=r2[:], in0=r[:], in1=r[:], op=Alu.mult)
            # sin poly
            S = [-1.6666667163e-01, 8.3333337680e-03, -1.9841270114e-04,
                 2.7557314297e-06, -2.5050759689e-08]
            p = spool.tile([1, n], f32, tag="trig_p")
            TS(out=p[:], in0=r2[:], scalar1=S[4], scalar2=S[3], op0=Alu.mult, op1=Alu.add)
            for cf in (S[2], S[1], S[0]):
                TT(out=p[:], in0=p[:], in1=r2[:], op=Alu.mult)
                TS(out=p[:], in0=p[:], scalar1=cf, scalar2=None, op0=Alu.add)
            r3 = spool.tile([1, n], f32, tag="trig_r3")
            TT(out=r3[:], in0=r2[:], in1=r[:], op=Alu.mult)
            sp = spool.tile([1, n], f32, tag="trig_sp")
            TT(out=sp[:], in0=p[:], in1=r3[:], op=Alu.mult)
            TT(out=sp[:], in0=sp[:], in1=r[:], op=Alu.add)
            # cos poly
            C = [4.1666667908e-02, -1.3888889225e-03, 2.4801587642e-05,
                 -2.7557314297e-07]
            cpl = spool.tile([1, n], f32, tag="trig_cp")
            TS(out=cpl[:], in0=r2[:], scalar1=C[3], scalar2=C[2], op0=Alu.mult, op1=Alu.add)
            for cf in (C[1], C[0]):
                TT(out=cpl[:], in0=cpl[:], in1=r2[:], op=Alu.mult)
                TS(out=cpl[:], in0=cpl[:], scalar1=cf, scalar2=None, op0=Alu.add)
            TT(out=cpl[:], in0=cpl[:], in1=r2[:], op=Alu.mult)
            TS(out=cpl[:], in0=cpl[:], scalar1=-0.5, scalar2=None, op0=Alu.add)
            TT(out=cpl[:], in0=cpl[:], in1=r2[:], op=Alu.mult)
            cp = spool.tile([1, n], f32, tag="trig_cpf")
            TS(out=cp[:], in0=cpl[:], scalar1=1.0, scalar2=None, op0=Alu.add)
            # quadrant select: qm = q + 4*(q<0) ; masks
            neg = spool.tile([1, n], f32, tag="trig_neg")
            TS(out=neg[:], in0=q[:], scalar1=0.0, scalar2=None, op0=Alu.is_lt)
            qm = spool.tile([1, n], f32, tag="trig_qm")
            STT(out=qm[:], in0=neg[:], scalar=4.0, in1=q[:], op0=Alu.mult, op1=Alu.add)
            sres = spool.tile([1, n], f32, tag="trig_sres")
            cres = spool.tile([1, n], f32, tag="trig_cres")
            m = spool.tile([1, n], f32, tag="trig_m")
            tm = spool.tile([1, n], f32, tag="trig_tm")
            # sin = m0*sp + m1*cp - m2*sp - m3*cp ; cos = m0*cp - m1*sp - m2*cp + m3*sp
            TS(out=m[:], in0=qm[:], scalar1=0.0, scalar2=None, op0=Alu.is_equal)
            TT(out=sres[:], in0=m[:], in1=sp[:], op=Alu.mult)
            TT(out=cres[:], in0=m[:], in1=cp[:], op=Alu.mult)
            TS(out=m[:], in0=qm[:], scalar1=1.0, scalar2=None, op0=Alu.is_equal)
            TT(out=tm[:], in0=m[:], in1=cp[:], op=Alu.mult)
            TT(out=sres[:], in0=sres[:], in1=tm[:], op=Alu.add)
            TT(out=tm[:], in0=m[:], in1=sp[:], op=Alu.mult)
            TT(out=cres[:], in0=cres[:], in1=tm[:], op=Alu.subtract)
            TS(out=m[:], in0=qm[:], scalar1=2.0, scalar2=None, op0=Alu.is_equal)
            TT(out=tm[:], in0=m[:], in1=sp[:], op=Alu.mult)
            TT(out=sres[:], in0=sres[:], in1=tm[:], op=Alu.subtract)
            TT(out=tm[:], in0=m[:], in1=cp[:], op=Alu.mult)
            TT(out=cres[:], in0=cres[:], in1=tm[:], op=Alu.subtract)
            TS(out=m[:], in0=qm[:], scalar1=3.0, scalar2=None, op0=Alu.is_equal)
            TT(out=tm[:], in0=m[:], in1=cp[:], op=Alu.mult)
            TT(out=sres[:], in0=sres[:], in1=tm[:], op=Alu.subtract)
            TT(out=tm[:], in0=m[:], in1=sp[:], op=Alu.mult)
            TT(out=cres[:], in0=cres[:], in1=tm[:], op=Alu.add)
            return sres, cres

        sinv, cosv = trig(rowsf)   # [1, 6]: cols = (img, angle)

        # ---- SIREN in [LAT, B_LOC] layout -------------------------------
        def sin_reduced(dst, z_sb, bias_col, scale):
            """dst = sin(scale*z + bias) with range reduction; all [LAT, B_LOC]."""
            t = spool.tile([LAT, B_LOC], f32, tag="sir_t")
            if bias_col is not None:
                TS(out=t[:], in0=z_sb[:], scalar1=bias_col, scalar2=float(scale),
                   op0=Alu.add, op1=Alu.mult)
            else:
                TS(out=t[:], in0=z_sb[:], scalar1=float(scale), scalar2=None, op0=Alu.mult)
            u = spool.tile([LAT, B_LOC], f32, tag="sir_u")
            TS(out=u[:], in0=t[:], scalar1=float(1.0 / (2 * np.pi)), scalar2=None, op0=Alu.mult)
            k = spool.tile([LAT, B_LOC], f32, tag="sir_k")
            TS(out=k[:], in0=u[:], scalar1=MAGIC, scalar2=MAGIC, op0=Alu.add, op1=Alu.subtract)
            r = spool.tile([LAT, B_LOC], f32, tag="sir_r")
            STT(out=r[:], in0=k[:], scalar=float(-2 * np.pi), in1=t[:], op0=Alu.mult, op1=Alu.add)
            nc.scalar.activation(out=dst[:], in_=r[:], func=Act.Sin)

        h = spool.tile([LAT, B_LOC], f32, tag="h")
        zp = sppool.tile([LAT, B_LOC], f32, tag="sp")
        nc.tensor.matmul(out=zp[:], lhsT=wm[:, 0:LAT], rhs=latT[:], start=True, stop=True)
        z_sb = spool.tile([LAT, B_LOC], f32, tag="z_sb")
        nc.scalar.copy(out=z_sb[:], in_=zp[:])
        sin_reduced(h, z_sb, bv[:, 0:1], W0_FIRST)
        for li in range(1, 4):
            zp2 = sppool.tile([LAT, B_LOC], f32, tag="sp")
            nc.tensor.matmul(out=zp2[:], lhsT=wm[:, li * LAT:(li + 1) * LAT],
                             rhs=h[:], start=True, stop=True)
            nc.scalar.copy(out=z_sb[:], in_=zp2[:])
            sn = spool.tile([LAT, B_LOC], f32, tag="sir_sn")
            sin_reduced(sn, z_sb, bv[:, li:li + 1], 1.0)
            TT(out=h[:], in0=h[:], in1=sn[:], op=Alu.add)

        # ---- assemble scalar row s [1, 16*B_LOC] ------------------------
        NSC = 16
        s = spool.tile([1, NSC * B_LOC], f32, tag="s_row")

        def ang(b_, k_):  # AP helpers into sinv/cosv columns
            return (3 * b_ + k_, 3 * b_ + k_ + 1)

        t1 = spool.tile([1, 1], f32, tag="t1")
        t2 = spool.tile([1, 1], f32, tag="t2")
        for b_ in range(B_LOC):
            o = NSC * b_
            ca = cosv[:, 3 * b_:3 * b_ + 1]; sa = sinv[:, 3 * b_:3 * b_ + 1]
            cb = cosv[:, 3 * b_ + 1:3 * b_ + 2]; sb = sinv[:, 3 * b_ + 1:3 * b_ + 2]
            cg = cosv[:, 3 * b_ + 2:3 * b_ + 3]; sg = sinv[:, 3 * b_ + 2:3 * b_ + 3]
            # R00 = cg*cb*ca - sg*sa
            TT(out=t1[:], in0=cg, in1=cb, op=Alu.mult)
            TT(out=t1[:], in0=t1[:], in1=ca, op=Alu.mult)
            TT(out=t2[:], in0=sg, in1=sa, op=Alu.mult)
            TT(out=s[:, o + 0:o + 1], in0=t1[:], in1=t2[:], op=Alu.subtract)
            # R01 = cg*cb*sa + sg*ca
            TT(out=t1[:], in0=cg, in1=cb, op=Alu.mult)
            TT(out=t1[:], in0=t1[:], in1=sa, op=Alu.mult)
            TT(out=t2[:], in0=sg, in1=ca, op=Alu.mult)
            TT(out=s[:, o + 1:o + 2], in0=t1[:], in1=t2[:], op=Alu.add)
            # R02 = -cg*sb
            TT(out=t1[:], in0=cg, in1=sb, op=Alu.mult)
            TS(out=s[:, o + 2:o + 3], in0=t1[:], scalar1=-1.0, scalar2=None, op0=Alu.mult)
            # sx + 128
            TS(out=s[:, o + 3:o + 4], in0=shf[:, 2 * b_:2 * b_ + 1],
               scalar1=float(XS // 2), scalar2=None, op0=Alu.add)
            # R10 = -(sg*cb*ca + cg*sa)
            TT(out=t1[:], in0=sg, in1=cb, op=Alu.mult)
            TT(out=t1[:], in0=t1[:], in1=ca, op=Alu.mult)
            TT(out=t2[:], in0=cg, in1=sa, op=Alu.mult)
            TT(out=t1[:], in0=t1[:], in1=t2[:], op=Alu.add)
            TS(out=s[:, o + 4:o + 5], in0=t1[:], scalar1=-1.0, scalar2=None, op0=Alu.mult)
            # R11 = cg*ca - sg*cb*sa
            TT(out=t1[:], in0=sg, in1=cb, op=Alu.mult)
            TT(out=t1[:], in0=t1[:], in1=sa, op=Alu.mult)
            TT(out=t2[:], in0=cg, in1=ca, op=Alu.mult)
            TT(out=s[:, o + 5:o + 6], in0=t2[:], in1=t1[:], op=Alu.subtract)
            # R12 = sg*sb
            TT(out=s[:, o + 6:o + 7], in0=sg, in1=sb, op=Alu.mult)
            # sy + 128
            TS(out=s[:, o + 7:o + 8], in0=shf[:, 2 * b_ + 1:2 * b_ + 2],
               scalar1=float(XS // 2), scalar2=None, op0=Alu.add)
            # h columns via transpose of h[:, b] -> [1, LAT]
            hp = sppool.tile([1, LAT], f32, tag="sp")
            nc.tensor.transpose(out=hp[:], in_=h[:, b_:b_ + 1], identity=idn[:LAT, :LAT])
            nc.vector.tensor_copy(out=s[:, o + 8:o + 16], in_=hp[:])

        # broadcast s across partitions
        bps = sppool.tile([P, NSC * B_LOC], f32, tag="sp")
        nc.tensor.matmul(out=bps[:], lhsT=onesr[:], rhs=s[:], start=True, stop=True)
        bc = spool.tile([P, NSC * B_LOC], f32, tag="bcast")
        nc.vector.tensor_copy(out=bc[:], in_=bps[:])
        if debug:
            nc.sync.dma_start(out=dbg_bc[:], in_=bc[:])

        def bcol(b_, k_):
            return bc[:, NSC * b_ + k_: NSC * b_ + k_ + 1]

        # ---- scatter accumulators ---------------------------------------
        img_ps = [ppool.tile([P, 512], f32, name=f"img_ps{_b}", tag="img_ps") for _b in range(B_LOC)]
        n_chunks_total = cols

        # ---- main loop ---------------------------------------------------
        done_chunks = 0
        for st in range(n_st):
            w = min(st_cols, cols - st * st_cols)
            inp = inpool.tile([P, 13, st_cols], f32, tag="inp")
            nc.sync.dma_start(out=inp[:, :, :w], in_=packed[:, :, st * st_cols: st * st_cols + w])
            cx = inp[:, 0, :w]; cy = inp[:, 1, :w]; cz = inp[:, 2, :w]
            vals_p = inp[:, 3, :w]; bd_p = inp[:, 4, :w]

            vb = plpool.tile([P, st_cols], f32, tag="vb")
            TT(out=vb[:, :w], in0=vals_p, in1=bd_p, op=Alu.add)

            lo16 = []; hi16 = []; v16 = []; nlo16 = []
            for b_ in range(B_LOC):
                # px = cx*R00 + cy*R01 + cz*R02 + (sx+128)
                px = plpool.tile([P, st_cols], f32, tag="px")
                TS(out=px[:, :w], in0=cx, scalar1=bcol(b_, 0), scalar2=None, op0=Alu.mult)
                STT(out=px[:, :w], in0=cy, scalar=bcol(b_, 1), in1=px[:, :w], op0=Alu.mult, op1=Alu.add)
                STT(out=px[:, :w], in0=cz, scalar=bcol(b_, 2), in1=px[:, :w], op0=Alu.mult, op1=Alu.add)
                TS(out=px[:, :w], in0=px[:, :w], scalar1=bcol(b_, 3), scalar2=None, op0=Alu.add)
                py = plpool.tile([P, st_cols], f32, tag="py")
                TS(out=py[:, :w], in0=cx, scalar1=bcol(b_, 4), scalar2=None, op0=Alu.mult)
                STT(out=py[:, :w], in0=cy, scalar=bcol(b_, 5), in1=py[:, :w], op0=Alu.mult, op1=Alu.add)
                STT(out=py[:, :w], in0=cz, scalar=bcol(b_, 6), in1=py[:, :w], op0=Alu.mult, op1=Alu.add)
                TS(out=py[:, :w], in0=py[:, :w], scalar1=bcol(b_, 7), scalar2=None, op0=Alu.add)
                # round + clip
                TS(out=px[:, :w], in0=px[:, :w], scalar1=MAGIC, scalar2=MAGIC, op0=Alu.add, op1=Alu.subtract)
                TS(out=px[:, :w], in0=px[:, :w], scalar1=0.0, scalar2=255.0, op0=Alu.max, op1=Alu.min)
                TS(out=py[:, :w], in0=py[:, :w], scalar1=MAGIC, scalar2=MAGIC, op0=Alu.add, op1=Alu.subtract)
                TS(out=py[:, :w], in0=py[:, :w], scalar1=0.0, scalar2=255.0, op0=Alu.max, op1=Alu.min)
                # hi = floor(py/2) = round(py*0.5 - 0.25) ; m = py - 2*hi ; lo = m*256 + px
                hi = plpool.tile([P, st_cols], f32, tag="hi")
                TS(out=hi[:, :w], in0=py[:, :w], scalar1=0.5, scalar2=-0.25, op0=Alu.mult, op1=Alu.add)
                TS(out=hi[:, :w], in0=hi[:, :w], scalar1=MAGIC, scalar2=MAGIC, op0=Alu.add, op1=Alu.subtract)
                m = plpool.tile([P, st_cols], f32, tag="m")
                STT(out=m[:, :w], in0=hi[:, :w], scalar=-2.0, in1=py[:, :w], op0=Alu.mult, op1=Alu.add)
                lo_t = ohpool.tile([P, st_cols], f32, tag="lo16", bufs=3)
                STT(out=lo_t[:, :w], in0=m[:, :w], scalar=256.0, in1=px[:, :w], op0=Alu.mult, op1=Alu.add)
                if ACT_RES:
                    nlo_t = ohpool.tile([P, st_cols], f32, tag="nlo16", bufs=3)
                    TS(out=nlo_t[:, :w], in0=lo_t[:, :w], scalar1=-1.0, scalar2=None, op0=Alu.mult)
                else:
                    nlo_t = None
                hi_t = ohpool.tile([P, st_cols], f32, tag="hi16", bufs=3)
                nc.vector.tensor_copy(out=hi_t[:, :w], in_=hi[:, :w])
                # v = vb + sum_l h[l]*Wd_l
                acc = plpool.tile([P, st_cols], f32, tag="acc")
                STT(out=acc[:, :w], in0=inp[:, 5, :w], scalar=bcol(b_, 8), in1=vb[:, :w],
                    op0=Alu.mult, op1=Alu.add)
                for l_ in range(1, LAT):
                    STT(out=acc[:, :w], in0=inp[:, 5 + l_, :w], scalar=bcol(b_, 8 + l_),
                        in1=acc[:, :w], op0=Alu.mult, op1=Alu.add)
                v_t = ohpool.tile([P, st_cols], f32, tag="v16", bufs=3)
                nc.vector.tensor_copy(out=v_t[:, :w], in_=acc[:, :w])
                lo16.append(lo_t); hi16.append(hi_t); v16.append(v_t)
                nlo16.append(nlo_t)
                if debug and st == 0 and b_ == 0:
                    dw = min(64, w)
                    nc.sync.dma_start(out=dbg_pl[0, :, :dw], in_=px[:, :dw])
                    nc.sync.dma_start(out=dbg_pl[1, :, :dw], in_=py[:, :dw])
                    nc.sync.dma_start(out=dbg_pl[2, :, :dw], in_=lo_t[:, :dw])
                    nc.sync.dma_start(out=dbg_pl[3, :, :dw], in_=hi_t[:, :dw])
                    nc.sync.dma_start(out=dbg_pl[4, :, :dw], in_=v_t[:, :dw])

            for c in range(w):
                first = (done_chunks == 0)
                last = (done_chunks == n_chunks_total - 1)
                # ACT engine builds the wide one-hot for a fraction of chunks
                # (relu(1 - |iota - lo|), exact at integers) to offload DVE
                on_act = (done_chunks % ACT_MOD) in ACT_RES
                on_pool = (done_chunks % ACT_MOD) in POOL_RES
                for b_ in range(B_LOC):
                    v_here = (not V_IN_OH1) and not on_act and not on_pool
                    oh5 = ohpool.tile([P, 512], f16, tag="oh5", bufs=OH_BUFS)
                    if on_act:
                        ab = ohpool.tile([P, 512], f16, tag="abs_t", bufs=max(4, OH_BUFS // 2))
                        nc.scalar.activation(out=ab[:], in_=io512[:], func=Act.Abs,
                                             bias=nlo16[b_][:, c:c + 1])
                        nc.scalar.activation(out=oh5[:], in_=ab[:], func=Act.Relu,
                                             bias=1.0, scale=-1.0)
                    elif on_pool:
                        nc.gpsimd.tensor_scalar(
                            out=oh5[:], in0=io512[:], scalar1=lo16[b_][:, c:c + 1],
                            scalar2=None, op0=Alu.is_equal)
                    elif v_here:
                        TS(out=oh5[:], in0=io512[:], scalar1=lo16[b_][:, c:c + 1],
                           scalar2=v16[b_][:, c:c + 1], op0=Alu.is_equal, op1=Alu.mult)
                    else:
                        TS(out=oh5[:], in0=io512[:], scalar1=lo16[b_][:, c:c + 1],
                           scalar2=None, op0=Alu.is_equal)
                    # v rides whichever one-hot doesn't come from ACT
                    oh1 = ohpool.tile([P, P], f16, tag="oh1", bufs=OH_BUFS)
                    if v_here:
                        TS(out=oh1[:], in0=io128[:], scalar1=hi16[b_][:, c:c + 1],
                           scalar2=None, op0=Alu.is_equal)
                    else:
                        TS(out=oh1[:], in0=io128[:], scalar1=hi16[b_][:, c:c + 1],
                           scalar2=v16[b_][:, c:c + 1], op0=Alu.is_equal, op1=Alu.mult)
                    nc.tensor.matmul(out=img_ps[b_][:], lhsT=oh1[:], rhs=oh5[:],
                                     start=first, stop=last, skip_group_check=True)
                done_chunks += 1

        # ---- post-processing per image ----------------------------------
        for b_ in range(B_LOC):
            img_sb = popool.tile([P, 512], f32, tag="img_sb")
            nc.scalar.copy(out=img_sb[:], in_=img_ps[b_][:])
            if debug:
                nc.sync.dma_start(out=dbg_img[b_], in_=img_sb[:])
            # Y-pass: Ty[c][kh] = sum_j A_c(j, kh)^T @ img[:, j*256:...]
            Ty = []
            for ci in range(2):           # 0: real, 1: imag
                tysb = popool.tile([P, 512], f32, tag=f"ty{ci}")
                for kh in range(2):
                    tp = pppool.tile([P, 256], f32, tag="pp")
                    for j in range(2):
                        nc.tensor.matmul(out=tp[:], lhsT=pmat(ci, j, kh),
                                         rhs=img_sb[:, j * 256:(j + 1) * 256],
                                         start=(j == 0), stop=(j == 1))
                    nc.scalar.copy(out=tysb[:, kh * 256:(kh + 1) * 256], in_=tp[:])
                Ty.append(tysb)
            # transpose Ty -> TyT [x-part, ky-free]
            TyT = []
            for ci in range(2):
                ttsb = popool.tile([P, 512], f32, tag=f"tyt{ci}")
                for kh in range(2):
                    for xh in range(2):
                        tp = pppool.tile([P, P], f32, tag="pp")
                        nc.tensor.transpose(
                            out=tp[:], in_=Ty[ci][:, kh * 256 + xh * 128: kh * 256 + (xh + 1) * 128],
                            identity=idn[:])
                        nc.scalar.copy(
                            out=ttsb[:, xh * 256 + kh * 128: xh * 256 + (kh + 1) * 128], in_=tp[:])
                TyT.append(ttsb)
            # X-pass: F[oc][kxh] ; Fr = Cr@Tr - Ci@Ti ; Fi = Cr@Ti + Ci@Tr
            # then G = F * ctf
            G = []
            for oc in range(2):
                gsb = popool.tile([P, 512], f32, tag=f"g{oc}")
                terms = ([(2, 0), (4, 1)] if oc == 0 else [(2, 1), (3, 0)])
                for kxh in range(2):
                    fp = pppool.tile([P, 256], f32, tag="pp")
                    mm = 0
                    for (mk, src) in terms:
                        for xh in range(2):
                            nc.tensor.matmul(out=fp[:], lhsT=pmat(mk, xh, kxh),
                                             rhs=TyT[src][:, xh * 256:(xh + 1) * 256],
                                             start=(mm == 0), stop=(mm == 3))
                            mm += 1
                    TT(out=gsb[:, kxh * 256:(kxh + 1) * 256], in0=fp[:],
                       in1=ctf_sb[:, b_ * 512 + kxh * 256: b_ * 512 + (kxh + 1) * 256],
                       op=Alu.mult)
                G.append(gsb)
            # iX-pass: Z[oc][xh] ; Zr = Vr@Gr - Vi@Gi ; Zi = Vr@Gi + Vi@Gr
            Z = []
            for oc in range(2):
                zsb = popool.tile([P, 512], f32, tag=f"z{oc}")
                terms = ([(5, 0), (7, 1)] if oc == 0 else [(5, 1), (6, 0)])
                for xh in range(2):
                    zp_ = pppool.tile([P, 256], f32, tag="pp")
                    mm = 0
                    for (mk, src) in terms:
                        for kxh in range(2):
                            nc.tensor.matmul(out=zp_[:], lhsT=pmat(mk, kxh, xh),
                                             rhs=G[src][:, kxh * 256:(kxh + 1) * 256],
                                             start=(mm == 0), stop=(mm == 3))
                            mm += 1
                    nc.scalar.copy(out=zsb[:, xh * 256:(xh + 1) * 256], in_=zp_[:])
                Z.append(zsb)
            # transpose Z -> Zt[oc][kyh] [ky-part, x-free]
            Zt = [[], []]
            for oc in range(2):
                for kyh in range(2):
                    ztsb = popool.tile([P, 256], f32, tag=f"zt{oc}{kyh}")
                    for xh in range(2):
                        tp = pppool.tile([P, P], f32, tag="pp")
                        nc.tensor.transpose(
                            out=tp[:], in_=Z[oc][:, xh * 256 + kyh * 128: xh * 256 + (kyh + 1) * 128],
                            identity=idn[:])
                        nc.scalar.copy(out=ztsb[:, xh * 128:(xh + 1) * 128], in_=tp[:])
                    Zt[oc].append(ztsb)
            # iY-pass: out[yh] = sum_kyh (Wy_r@Ztr - Wy_i@Zti)
            for yh in range(2):
                op_ = pppool.tile([P, 256], f32, tag="pp")
                mm = 0
                for (mk, oc) in ((8, 0), (9, 1)):
                    for kyh in range(2):
                        nc.tensor.matmul(out=op_[:], lhsT=pmat(mk, kyh, yh),
                                         rhs=Zt[oc][kyh][:], start=(mm == 0), stop=(mm == 3))
                        mm += 1
                osb = popool.tile([P, 256], f32, tag="osb")
                nc.scalar.copy(out=osb[:], in_=op_[:])
                nc.sync.dma_start(out=out[b_, yh * P:(yh + 1) * P, :], in_=osb[:])

    _split_sync_waits(nc)
    return nc


# ---------------------------------------------------------------------------
# host wrapper
# ---------------------------------------------------------------------------

_PROG_CACHE = {}


def _get_program(cols, debug=False):
    key = (cols, debug)
    if key not in _PROG_CACHE:
        _PROG_CACHE[key] = build_program(cols=cols, debug=debug)
    return _PROG_CACHE[key]


def _marshal(inputs, cols=COLS):
    npad = P * cols
    n_use = min(N, npad)

    def plane(a):
        f = np.zeros(npad, np.float32)
        f[:n_use] = np.asarray(a, np.float32).ravel()[:n_use]
        return f.reshape(P, cols)

    coords = np.asarray(inputs["coords"], np.float32)
    packed = np.zeros((P, 13, cols), np.float32)
    packed[:, 0] = plane(coords[:n_use, 0])
    packed[:, 1] = plane(coords[:n_use, 1])
    packed[:, 2] = plane(coords[:n_use, 2])
    packed[:, 3] = plane(inputs["values"][:n_use])
    packed[:, 4] = plane(inputs["bd"][:n_use])
    Wd = np.asarray(inputs["Wd"], np.float32)
    for l_ in range(LAT):
        packed[:, 5 + l_] = plane(Wd[l_, :n_use])

    iota512 = np.tile(np.arange(512, dtype=np.float16), (P, 1))
    iota128 = np.tile(np.arange(P, dtype=np.float16), (P, 1))
    ones_row = np.ones((1, P), np.float32)
    ident = np.eye(P, dtype=np.float32)
    postmats = np.ascontiguousarray(_post_matrices().transpose(1, 0, 2))  # [128, 10, 512]
    wmats = np.concatenate([np.asarray(inputs[k], np.float32) for k in ("W0", "W1", "W2", "W3")], axis=1)
    bvecs = np.stack([np.asarray(inputs[k], np.float32) for k in ("b0", "b1", "b2", "b3")], axis=1)

    rows = np.asarray(inputs["rows"], np.float32)
    shifts = np.asarray(inputs["shifts"], np.float32)
    latent = np.asarray(inputs["latent"], np.float32)
    ctf = np.asarray(inputs["ctf"], np.float32)

    in_maps = []
    for core in range(N_CORES):
        bs = slice(core * B_LOC, (core + 1) * B_LOC)
        ctf_core = np.concatenate(
            [_ctf_extend_T(c).reshape(P, 512) for c in ctf[bs]], axis=1)
        in_maps.append({
            "packed": packed,
            "rows_flat": rows[bs].reshape(1, 6),
            "shifts_flat": shifts[bs].reshape(1, 4),
            "latentT": np.ascontiguousarray(latent[bs].T),
            "wmats": wmats,
            "bvecs": np.ascontiguousarray(bvecs),
            "ctf_arr": ctf_core,
            "iota512": iota512,
            "iota128": iota128,
            "ones_row": ones_row,
            "ident": ident,
            "postmats": postmats,
        })
    return in_maps


def run(inputs, cols=COLS, trace=False, debug=False):
    from concourse.bass_utils import run_bass_kernel_spmd
    nc = _get_program(cols, debug)
    in_maps = _marshal(inputs, cols)
    res = run_bass_kernel_spmd(nc, in_maps, list(range(N_CORES)), trace=trace)
    outs = [res.results[i]["out"] for i in range(N_CORES)]
    full = np.concatenate(outs, axis=0).astype(np.float32)
    return full, res


def kernel(**inputs):
    out, _ = run(inputs)
    return out

